# revision 1
# baseline (speedup 1.0000x reference)
"""Trainium2 Bass kernel for nn_MnistNet (ternary-weight MLP with training-mode
BatchNorm), data-parallel over batch across 8 NeuronCores.

Strategy
--------
- Host side does layout-only prep: transpose x / weights, zero-pad 784->896,
  shard the batch (1024 rows/core) and the weight rows (1/8 per core).
- Device side does all math:
  * ternarize: t = Sign(w/delta - 1) + Sign(w/delta + 1)  in {-2, 0, +2}.
    The 2x scale is exactly washed out by the following BatchNorm (scale
    invariance; eps distortion ~1e-9).  delta = 0.7*mean|W| via per-core
    partial |.| row-sums + one tiny AllReduce + a ones-matmul that both
    finishes the partition reduction and broadcasts the result.
  * biases b_in/b_hid/b_out are dropped entirely: BatchNorm subtracts the
    per-feature mean, so any per-feature constant shift cancels exactly.
  * each core ternarizes 1/8 of each hidden/output weight matrix into a tiled
    bf16 layout and AllGathers the result (weights exactly representable).
  * matmuls in bf16 (fp32 PSUM accumulation), activations kept transposed
    (features on partitions, batch on the free dim) so BN stats are free-dim
    reductions fused into the PSUM-drain ops (accum_out).
  * per-layer BN: 32KB AllReduce of (sum, sumsq), then a fused
    scale/bias ACT pass + min/max DVE clamp (hardtanh).
  * log-softmax: exp on ACT, partition-sum via a ones-matmul, Ln, subtract.
"""

import os

import numpy as np

N_CORES = 8
B = 8192
B_LOC = B // N_CORES          # 1024 rows per core
HID = 4096
N_MID = 4
KIN = 784
KIN_PAD = 896                 # 7 * 128
KT_IN = KIN_PAD // 128        # 7
KT_H = HID // 128             # 32
MT = HID // 128               # 32 output-feature tiles
KL = 4                        # k-tiles per core shard of a hidden layer
SH_H = 512                    # hidden-weight rows per core (4096/8)
EPS = 1e-5
RG = [list(range(N_CORES))]

_cache = {}


def _build():
    n_mid_eff = int(os.environ.get("KERNEL_NMID", str(N_MID)))
    cut = int(os.environ.get("KERNEL_CUT", "0"))
    import concourse.bass as bass
    import concourse.bacc as bacc
    import concourse.mybir as mybir
    import concourse.tile as tile

    f32 = mybir.dt.float32
    bf16 = mybir.dt.bfloat16
    AX = mybir.AxisListType
    OP = mybir.AluOpType
    AF = mybir.ActivationFunctionType

    nc = bacc.Bacc("TRN2", target_bir_lowering=False, debug=False,
                   num_devices=N_CORES)

    # ---- I/O ----------------------------------------------------------------
    xT = nc.dram_tensor("xT", [KIN_PAD, B_LOC], f32, kind="ExternalInput")
    winT = nc.dram_tensor("winT", [KIN_PAD, HID], f32, kind="ExternalInput")
    whT = nc.dram_tensor("whT", [N_MID, SH_H, HID], f32, kind="ExternalInput")
    woT = nc.dram_tensor("woT", [SH_H, 16], f32, kind="ExternalInput")
    gam = nc.dram_tensor("gam", [N_MID + 1, HID], f32, kind="ExternalInput")
    bet = nc.dram_tensor("bet", [N_MID + 1, HID], f32, kind="ExternalInput")
    gob = nc.dram_tensor("gob", [2, 10], f32, kind="ExternalInput")
    # [10, B_LOC] layout: a transposed DRAM store (interleaved partition
    # windows) reliably kills the device, so transpose on host instead.
    out = nc.dram_tensor("out", [10, B_LOC], f32, kind="ExternalOutput")

    with tile.TileContext(nc) as tc:
        with (
            tc.tile_pool(name="ht", bufs=1) as ht,
            tc.tile_pool(name="wmp", bufs=2) as wmp,
            tc.tile_pool(name="slab", bufs=2) as slabp,
            tc.tile_pool(name="tern", bufs=2) as ternp,
            tc.tile_pool(name="sq", bufs=2) as sqp,
            tc.tile_pool(name="small", bufs=1) as small,
            tc.tile_pool(name="stats", bufs=2) as stats,
            tc.tile_pool(name="psum", bufs=2, space="PSUM") as psum,
            tc.tile_pool(name="dram", bufs=1, space="DRAM") as dram,
        ):
            # ---- persistent small tiles ------------------------------------
            ones128 = small.tile([128, 128], f32, tag="ones128")
            nc.vector.memset(ones128, 1.0)
            ones10 = small.tile([10, 1], f32, tag="ones10")
            nc.vector.memset(ones10, 1.0)
            negone = small.tile([128, 1], f32, tag="negone")
            nc.vector.memset(negone, -1.0)
            posone = small.tile([128, 1], f32, tag="posone")
            nc.vector.memset(posone, 1.0)
            epsb = small.tile([128, 1], f32, tag="epsb")
            nc.vector.memset(epsb, EPS)

            gam_sb, bet_sb = [], []
            for l in range(N_MID + 1):
                g = small.tile([128, MT], f32, tag=f"gam{l}", name=f"gam_sb{l}")
                nc.gpsimd.dma_start(out=g, in_=gam[l].rearrange("(m p) -> p m", p=128))
                gam_sb.append(g)
                b = small.tile([128, MT], f32, tag=f"bet{l}", name=f"bet_sb{l}")
                nc.gpsimd.dma_start(out=b, in_=bet[l].rearrange("(m p) -> p m", p=128))
                bet_sb.append(b)
            go_sb = small.tile([10, 1], f32, tag="go")
            nc.gpsimd.dma_start(out=go_sb, in_=gob[0:1, :].rearrange("a f -> f a"))
            bo_sb = small.tile([10, 1], f32, tag="bo")
            nc.gpsimd.dma_start(out=bo_sb, in_=gob[1:2, :].rearrange("a f -> f a"))

            # ---- activation double buffers (transposed: [feat 128, batch]) --
            A = [ht.tile([128, B_LOC], bf16, tag=f"A{k}", name=f"htA{k}")
                 for k in range(KT_H)]
            Bt = [ht.tile([128, B_LOC], bf16, tag=f"B{k}", name=f"htB{k}")
                  for k in range(KT_H)]

            # ---- DRAM scratch ----------------------------------------------
            tw_in = dram.tile([MT, KT_IN, 128, 128], bf16)
            tw_hid_sh = dram.tile([N_MID, MT, KL, 128, 128], bf16)
            tw_hid = [dram.tile([N_CORES, MT, KL, 128, 128], bf16,
                                addr_space="Shared", tag=f"tw_hid{l}",
                                name=f"tw_hid{l}")
                      for l in range(N_MID)]
            tw_out_sh = dram.tile([SH_H, 16], bf16)
            tw_out = dram.tile([HID, 16], bf16, addr_space="Shared")
            dlA_in = dram.tile([128, 1], f32)
            dlA_out = dram.tile([128, 1], f32, addr_space="Shared")
            dlB_in = dram.tile([128, 4], f32)
            dlB_out = dram.tile([128, 4], f32, addr_space="Shared")

            # ---- helpers ----------------------------------------------------
            def bcast_delta(partial_col, n_elems, nm):
                """[128,1] per-partition partial |W| sums -> broadcasted
                1/delta [128,1] (all partitions equal)."""
                ps = psum.tile([128, 1], f32, tag="small", name=f"dps_{nm}",
                               bufs=1)
                nc.tensor.matmul(ps, ones128, partial_col, start=True, stop=True)
                dsc = small.tile([128, 1], f32, tag=f"dsc_{nm}")
                nc.scalar.activation(out=dsc, in_=ps, func=AF.Copy,
                                     scale=0.7 / float(n_elems))
                inv = small.tile([128, 1], f32, tag=f"inv_{nm}")
                nc.vector.reciprocal(out=inv, in_=dsc)
                return inv

            def tern_slab(src_ap, inv_ap, dst_ap, cols):
                """ternarize one [128, cols] f32 slab -> {-2,0,2} bf16 in DRAM.
                dst_ap must be a [128, cols//128, 128] view."""
                sl = slabp.tile([128, cols], f32, tag="slab", name="tslab")
                nc.sync.dma_start(out=sl, in_=src_ap)
                u = ternp.tile([128, cols], bf16, tag="u", name="ternu")
                v = ternp.tile([128, cols], bf16, tag="v", name="ternv")
                nc.scalar.activation(out=u, in_=sl, func=AF.Sign,
                                     bias=negone, scale=inv_ap)
                nc.scalar.activation(out=v, in_=sl, func=AF.Sign,
                                     bias=posone, scale=inv_ap)
                nc.vector.tensor_tensor(out=u, in0=u, in1=v, op=OP.add)
                nc.sync.dma_start(out=dst_ap,
                                  in_=u.rearrange("p (m c) -> p m c", c=128))

            def delta_reduce(src_slabs, n_slabs, nm):
                """abs row-sum partials of a list of slab APs -> [128,1]."""
                part = small.tile([128, 16], f32, tag=f"part_{nm}")
                nc.vector.memset(part, 0.0)
                for s, (ap, cols) in enumerate(src_slabs):
                    sl = slabp.tile([128, cols], f32, tag="slab", name="dslab")
                    nc.sync.dma_start(out=sl, in_=ap)
                    nc.vector.tensor_reduce(out=part[:, s:s + 1], in_=sl,
                                            axis=AX.X, op=OP.add,
                                            apply_absolute_value=True)
                tot = small.tile([128, 1], f32, tag=f"ptot_{nm}")
                nc.vector.tensor_reduce(out=tot, in_=part, axis=AX.X, op=OP.add)
                return tot

            # background work queue: thunks emitted interleaved into m-loops
            bg = []

            def pump(n=1):
                for _ in range(min(n, len(bg))):
                    bg.pop(0)()

            # ---- layer runner ----------------------------------------------
            def mm_layer(lname, ht_in, n_kt, ht_out, w_read, gam_l, bet_l,
                         wm_shape, wm_slice):
                S1 = stats.tile([128, MT], f32, tag="s1", name=f"S1_{lname}")
                S2 = stats.tile([128, MT], f32, tag="s2", name=f"S2_{lname}")
                for m in range(MT):
                    wm = wmp.tile(wm_shape, bf16, tag="wm",
                                  name=f"wm_{lname}_{m}")
                    w_read(m, wm)
                    ps = psum.tile([128, B_LOC], f32, tag="mm",
                                   name=f"ps_{lname}_{m}")
                    for n in range(2):
                        for k in range(n_kt):
                            nc.tensor.matmul(
                                ps[:, n * 512:(n + 1) * 512],
                                wm_slice(wm, k),
                                ht_in[k][:, n * 512:(n + 1) * 512],
                                start=(k == 0), stop=(k == n_kt - 1))
                    nc.vector.tensor_scalar(
                        out=ht_out[m], in0=ps, scalar1=1.0, scalar2=None,
                        op0=OP.mult, op1=OP.add, accum_out=S1[:, m:m + 1])
                    sj = sqp.tile([128, B_LOC], bf16, tag="sq", name="sqj")
                    nc.scalar.activation(out=sj, in_=ps, func=AF.Square,
                                         accum_out=S2[:, m:m + 1])
                    pump(2)
                pump(len(bg))
                # BN stats allreduce
                bin_ = dram.tile([128, 64], f32, tag=f"bns_in_{lname}",
                                 name=f"bns_in_{lname}")
                bout_ = dram.tile([128, 64], f32, addr_space="Shared",
                                  tag=f"bns_out_{lname}", name=f"bns_out_{lname}")
                nc.gpsimd.dma_start(out=bin_[:, 0:32], in_=S1)
                nc.gpsimd.dma_start(out=bin_[:, 32:64], in_=S2)
                nc.gpsimd.collective_compute(
                    "AllReduce", OP.add, replica_groups=RG,
                    ins=[bin_.opt()], outs=[bout_.opt()])
                sg = stats.tile([128, 64], f32, tag="sg", name=f"sg_{lname}")
                nc.gpsimd.dma_start(out=sg, in_=bout_)
                # scale = gamma * rsqrt(var+eps); bias = beta - mean*scale
                mean = stats.tile([128, MT], f32, tag="mean", name=f"mean_{lname}")
                nc.vector.tensor_scalar_mul(mean, sg[:, 0:32], 1.0 / B)
                ex2 = stats.tile([128, MT], f32, tag="ex2", name=f"ex2_{lname}")
                nc.vector.tensor_scalar_mul(ex2, sg[:, 32:64], 1.0 / B)
                msq = stats.tile([128, MT], f32, tag="msq", name=f"msq_{lname}")
                nc.vector.tensor_tensor(out=msq, in0=mean, in1=mean, op=OP.mult)
                var = stats.tile([128, MT], f32, tag="var", name=f"var_{lname}")
                nc.vector.tensor_tensor(out=var, in0=ex2, in1=msq, op=OP.subtract)
                sd = stats.tile([128, MT], f32, tag="sd", name=f"sd_{lname}")
                nc.scalar.activation(out=sd, in_=var, func=AF.Sqrt, bias=epsb)
                rs = stats.tile([128, MT], f32, tag="rs", name=f"rs_{lname}")
                nc.vector.reciprocal(out=rs, in_=sd)
                scl = stats.tile([128, MT], f32, tag="scl", name=f"scl_{lname}")
                nc.vector.tensor_tensor(out=scl, in0=rs, in1=gam_l, op=OP.mult)
                mscl = stats.tile([128, MT], f32, tag="mscl", name=f"mscl_{lname}")
                nc.vector.tensor_tensor(out=mscl, in0=mean, in1=scl, op=OP.mult)
                bia = stats.tile([128, MT], f32, tag="bia", name=f"bia_{lname}")
                nc.vector.tensor_tensor(out=bia, in0=bet_l, in1=mscl,
                                        op=OP.subtract)
                # normalize + hardtanh, in k order for next-layer pipelining
                for k in range(MT):
                    nc.scalar.activation(out=ht_out[k], in_=ht_out[k],
                                         func=AF.Identity,
                                         bias=bia[:, k:k + 1],
                                         scale=scl[:, k:k + 1])
                    nc.vector.tensor_scalar(
                        out=ht_out[k], in0=ht_out[k], scalar1=1.0, scalar2=-1.0,
                        op0=OP.min, op1=OP.max)

            # ================= startup ======================================
            # x load + cast (feeds input-layer matmuls)
            xv = xT.rearrange("(t p) b -> t p b", p=128)
            for k in range(KT_IN):
                xs = slabp.tile([128, B_LOC], f32, tag="slab", name=f"xs{k}")
                nc.sync.dma_start(out=xs, in_=xv[k])
                nc.vector.tensor_copy(out=A[k], in_=xs)

            # delta + ternarize W_in (local, full matrix on every core)
            wv_in = winT.rearrange("(t p) f -> t p f", p=128)
            in_slabs = [(wv_in[t][:, h * 2048:(h + 1) * 2048], 2048)
                        for t in range(KT_IN) for h in range(2)]
            pin = delta_reduce(in_slabs, len(in_slabs), "in")
            inv_in = bcast_delta(pin, KIN * HID, "in")
            for t in range(KT_IN):
                for h in range(2):
                    tern_slab(wv_in[t][:, h * 2048:(h + 1) * 2048], inv_in,
                              tw_in[h * 16:(h + 1) * 16, t].rearrange(
                                  "m p c -> p m c"),
                              2048)

            # delta for hid0 (shard) -> AllReduce #1
            wv_h = [whT[l].rearrange("(kl p) f -> kl p f", p=128)
                    for l in range(N_MID)]
            if n_mid_eff > 0:
                h0_slabs = [(wv_h[0][kl][:, h * 2048:(h + 1) * 2048], 2048)
                            for kl in range(KL) for h in range(2)]
                ph0 = delta_reduce(h0_slabs, len(h0_slabs), "h0")
                nc.gpsimd.dma_start(out=dlA_in, in_=ph0)
                nc.gpsimd.collective_compute(
                    "AllReduce", OP.add, replica_groups=RG,
                    ins=[dlA_in.opt()], outs=[dlA_out.opt()])
                ph0g = small.tile([128, 1], f32, tag="ph0g")
                nc.gpsimd.dma_start(out=ph0g, in_=dlA_out)
                inv_h0 = bcast_delta(ph0g, HID * HID, "h0")

            # ternarize hid0 shard + AllGather (runs during input layer)
            def emit_tern_hid(l, inv):
                for kl in range(KL):
                    for h in range(2):
                        bg.append(lambda l=l, kl=kl, h=h, inv=inv: tern_slab(
                            wv_h[l][kl][:, h * 2048:(h + 1) * 2048], inv,
                            tw_hid_sh[l, h * 16:(h + 1) * 16, kl].rearrange(
                                "m p c -> p m c"),
                            2048))
                bg.append(lambda l=l: nc.gpsimd.collective_compute(
                    "AllGather", OP.bypass, replica_groups=RG,
                    ins=[tw_hid_sh[l].opt()], outs=[tw_hid[l].opt()]))

            if n_mid_eff > 0:
                emit_tern_hid(0, inv_h0)

            # delta partials for hid1..3 + out -> AllReduce #2 (as bg work)
            invs = {}

            def emit_delta_rest():
                pb = small.tile([128, 4], f32, tag="pb")
                nc.vector.memset(pb, 0.0)
                for i, l in enumerate(range(1, n_mid_eff)):
                    slabs = [(wv_h[l][kl][:, h * 2048:(h + 1) * 2048], 2048)
                             for kl in range(KL) for h in range(2)]
                    p = delta_reduce(slabs, len(slabs), f"h{l}")
                    nc.vector.tensor_copy(out=pb[:, i:i + 1], in_=p)
                wv_o = woT.rearrange("(s p) c -> s p c", p=128)
                o_slabs = [(wv_o[s], 16) for s in range(4)]
                po = delta_reduce(o_slabs, len(o_slabs), "out")
                nc.vector.tensor_copy(out=pb[:, 3:4], in_=po)
                nc.gpsimd.dma_start(out=dlB_in, in_=pb)
                nc.gpsimd.collective_compute(
                    "AllReduce", OP.add, replica_groups=RG,
                    ins=[dlB_in.opt()], outs=[dlB_out.opt()])
                pbg = small.tile([128, 4], f32, tag="pbg")
                nc.gpsimd.dma_start(out=pbg, in_=dlB_out)
                for i, l in enumerate(range(1, n_mid_eff)):
                    invs[l] = bcast_delta(pbg[:, i:i + 1], HID * HID, f"h{l}")
                invs["out"] = bcast_delta(pbg[:, 3:4], 10 * HID, "out")

            bg.append(emit_delta_rest)

            # ================= layers =======================================
            def w_read_in(m, wm):
                nc.sync.dma_start(out=wm,
                                  in_=tw_in[m].rearrange("k p c -> p k c"))

            def w_read_hid(l):
                def f(m, wm):
                    # per-rank reads: each is one contiguous 128KB block
                    for r in range(N_CORES):
                        nc.sync.dma_start(
                            out=wm[:, r, :, :],
                            in_=tw_hid[l][r, m].rearrange("kl p c -> p kl c"))
                return f

            bufs = [A, Bt]

            def emit_tern_out():
                wv_o2 = woT.rearrange("(s p) c -> s p c", p=128)
                tv = tw_out_sh.rearrange("(s p) c -> s p c", p=128)
                for s in range(4):
                    sl = slabp.tile([128, 16], f32, tag="slab",
                                    name="oslab")
                    nc.gpsimd.dma_start(out=sl, in_=wv_o2[s])
                    u = ternp.tile([128, 16], bf16, tag="u", name="ou")
                    v = ternp.tile([128, 16], bf16, tag="v", name="ov")
                    nc.scalar.activation(out=u, in_=sl, func=AF.Sign,
                                         bias=negone, scale=invs["out"])
                    nc.scalar.activation(out=v, in_=sl, func=AF.Sign,
                                         bias=posone, scale=invs["out"])
                    nc.vector.tensor_tensor(out=u, in0=u, in1=v, op=OP.add)
                    nc.gpsimd.dma_start(out=tv[s], in_=u)
                nc.gpsimd.collective_compute(
                    "AllGather", OP.bypass, replica_groups=RG,
                    ins=[tw_out_sh.opt()], outs=[tw_out.opt()])

            if n_mid_eff == 0:
                bg.append(emit_tern_out)
            done = False
            if cut == 1:
                pump(len(bg))
                fz = small.tile([10, B_LOC], f32, tag="fz")
                nc.vector.memset(fz, 0.0)
                nc.gpsimd.dma_start(out=out[:], in_=fz)
                done = True
            if not done:
                mm_layer("L0", A, KT_IN, Bt, w_read_in, gam_sb[0],
                         bet_sb[0], [128, KT_IN, 128],
                         lambda wm, k: wm[:, k, :])
            if cut == 2 and not done:
                fz = small.tile([10, B_LOC], f32, tag="fz")
                nc.vector.tensor_copy(out=fz, in_=Bt[0][0:10, :])
                nc.gpsimd.dma_start(out=out[:], in_=fz)
                done = True

            for l in range(n_mid_eff if not done else 0):
                ht_in = bufs[(l + 1) % 2]
                ht_out = bufs[l % 2]
                # queue ternarize of the NEXT hidden layer (or out layer)
                if l + 1 < n_mid_eff:
                    emit_tern_hid(l + 1, invs[l + 1])
                else:
                    bg.append(emit_tern_out)
                mm_layer(f"H{l}", ht_in, KT_H, ht_out, w_read_hid(l),
                         gam_sb[l + 1], bet_sb[l + 1],
                         [128, N_CORES, KL, 128],
                         lambda wm, k: wm[:, k // KL, k % KL, :])

            # ================= output layer + log-softmax ===================
            if not done:
                ht_fin = bufs[(n_mid_eff - 1) % 2]
                wmo = wmp.tile([128, KT_H, 16], bf16, tag="wm", name="wmo")
                nc.sync.dma_start(out=wmo,
                                  in_=tw_out.rearrange("(t p) c -> p t c", p=128))
                pso = psum.tile([10, B_LOC], f32, tag="mm", name="pso")
                for n in range(2):
                    for k in range(KT_H):
                        nc.tensor.matmul(
                            pso[:, n * 512:(n + 1) * 512],
                            wmo[:, k, 0:10],
                            ht_fin[k][:, n * 512:(n + 1) * 512],
                            start=(k == 0), stop=(k == KT_H - 1))
                S1o = stats.tile([10, 1], f32, tag="s1o")
                S2o = stats.tile([10, 1], f32, tag="s2o")
                opre = small.tile([10, B_LOC], f32, tag="opre")
                nc.vector.tensor_scalar(out=opre, in0=pso, scalar1=1.0,
                                        scalar2=None, op0=OP.mult, op1=OP.add,
                                        accum_out=S1o)
                sjo = sqp.tile([10, B_LOC], bf16, tag="sq", name="sqo")
                nc.scalar.activation(out=sjo, in_=pso, func=AF.Square,
                                     accum_out=S2o)
                bno_in = dram.tile([10, 2], f32)
                bno_out = dram.tile([10, 2], f32, addr_space="Shared")
                s12o = stats.tile([10, 2], f32, tag="s12o")
                nc.vector.tensor_copy(out=s12o[:, 0:1], in_=S1o)
                nc.vector.tensor_copy(out=s12o[:, 1:2], in_=S2o)
                nc.gpsimd.dma_start(out=bno_in, in_=s12o)
                nc.gpsimd.collective_compute(
                    "AllReduce", OP.add, replica_groups=RG,
                    ins=[bno_in.opt()], outs=[bno_out.opt()])
                sgo = stats.tile([10, 2], f32, tag="sgo")
                nc.gpsimd.dma_start(out=sgo, in_=bno_out)
                meano = stats.tile([10, 1], f32, tag="meano")
                nc.vector.tensor_scalar_mul(meano, sgo[:, 0:1], 1.0 / B)
                ex2o = stats.tile([10, 1], f32, tag="ex2o")
                nc.vector.tensor_scalar_mul(ex2o, sgo[:, 1:2], 1.0 / B)
                msqo = stats.tile([10, 1], f32, tag="msqo")
                nc.vector.tensor_tensor(out=msqo, in0=meano, in1=meano, op=OP.mult)
                varo = stats.tile([10, 1], f32, tag="varo")
                nc.vector.tensor_tensor(out=varo, in0=ex2o, in1=msqo,
                                        op=OP.subtract)
                sdo = stats.tile([10, 1], f32, tag="sdo")
                nc.scalar.activation(out=sdo, in_=varo, func=AF.Sqrt,
                                     bias=epsb[0:10, :])
                rso = stats.tile([10, 1], f32, tag="rso")
                nc.vector.reciprocal(out=rso, in_=sdo)
                sclo = stats.tile([10, 1], f32, tag="sclo")
                nc.vector.tensor_tensor(out=sclo, in0=rso, in1=go_sb, op=OP.mult)
                mso = stats.tile([10, 1], f32, tag="mso")
                nc.vector.tensor_tensor(out=mso, in0=meano, in1=sclo, op=OP.mult)
                biao = stats.tile([10, 1], f32, tag="biao")
                nc.vector.tensor_tensor(out=biao, in0=bo_sb, in1=mso,
                                        op=OP.subtract)
                onorm = small.tile([10, B_LOC], f32, tag="onorm")
                nc.scalar.activation(out=onorm, in_=opre, func=AF.Identity,
                                     bias=biao, scale=sclo)
                esb = small.tile([10, B_LOC], f32, tag="esb")
                nc.scalar.activation(out=esb, in_=onorm, func=AF.Exp)
                csp = psum.tile([1, B_LOC], f32, tag="cs", bufs=1)
                for n in range(2):
                    nc.tensor.matmul(csp[:, n * 512:(n + 1) * 512], ones10,
                                     esb[:, n * 512:(n + 1) * 512],
                                     start=True, stop=True)
                lsb = small.tile([1, B_LOC], f32, tag="lsb")
                nc.scalar.activation(out=lsb, in_=csp, func=AF.Ln)
                lrow = dram.tile([1, B_LOC], f32)
                nc.gpsimd.dma_start(out=lrow, in_=lsb)
                lr = lrow[0:1, :]
                lb_ap = bass.AP(tensor=lr.tensor, offset=lr.offset,
                                ap=[[0, 10], list(lr.ap[-1])])
                lb = small.tile([10, B_LOC], f32, tag="opre", name="lb")
                nc.gpsimd.dma_start(out=lb, in_=lb_ap)
                fout = small.tile([10, B_LOC], f32, tag="esb", name="fout")
                nc.vector.tensor_tensor(out=fout, in0=onorm, in1=lb,
                                        op=OP.subtract)
                nc.gpsimd.dma_start(out=out[:], in_=fout)

    nc.compile()
    return nc


def _get_program():
    if "nc" not in _cache:
        _cache["nc"] = _build()
    return _cache["nc"]


def kernel(x, W_in, b_in, W_hid, b_hid, W_out, b_out, gamma, beta,
           gamma_out, beta_out):
    from concourse.bass_utils import run_bass_kernel_spmd

    nc = _get_program()

    x = np.asarray(x, dtype=np.float32).reshape(B, KIN)
    # layout-only host prep (transpose + zero-pad + shard)
    xT_full = np.zeros((KIN_PAD, B), dtype=np.float32)
    xT_full[:KIN] = x.T
    winT_full = np.zeros((KIN_PAD, HID), dtype=np.float32)
    winT_full[:KIN] = np.asarray(W_in, dtype=np.float32).T
    whT_full = np.ascontiguousarray(
        np.asarray(W_hid, dtype=np.float32).transpose(0, 2, 1))
    woT_full = np.zeros((HID, 16), dtype=np.float32)
    woT_full[:, :10] = np.asarray(W_out, dtype=np.float32).T
    gam_np = np.ascontiguousarray(np.asarray(gamma, dtype=np.float32))
    bet_np = np.ascontiguousarray(np.asarray(beta, dtype=np.float32))
    gob_np = np.stack([np.asarray(gamma_out, dtype=np.float32),
                       np.asarray(beta_out, dtype=np.float32)])

    in_maps = []
    for c in range(N_CORES):
        in_maps.append({
            "xT": np.ascontiguousarray(
                xT_full[:, c * B_LOC:(c + 1) * B_LOC]),
            "winT": winT_full,
            "whT": np.ascontiguousarray(
                whT_full[:, c * SH_H:(c + 1) * SH_H, :]),
            "woT": np.ascontiguousarray(
                woT_full[c * SH_H:(c + 1) * SH_H, :]),
            "gam": gam_np,
            "bet": bet_np,
            "gob": gob_np,
        })

    res = run_bass_kernel_spmd(nc, in_maps, core_ids=list(range(N_CORES)))
    return np.concatenate(
        [np.ascontiguousarray(res.results[c]["out"].T) for c in range(N_CORES)],
        axis=0)



# revision 7
# speedup vs baseline: 1.0077x; 1.0077x over previous
"""Trainium2 Bass kernel for nn_MnistNet (ternary-weight MLP with training-mode
BatchNorm), data-parallel over batch across 8 NeuronCores.

Strategy (v2: fp8 DoubleRow)
----------------------------
- Host side does layout-only prep: transpose x / weights, zero-pad 784->1024,
  shard the batch (1024 rows/core) and the weight rows (1/8 per core).
- All matmuls run in fp8e4 with MatmulPerfMode.DoubleRow (2 k-tiles per
  instruction, 0.5 cycles/row -> up to 4x bf16 PE throughput).  Ternary
  weights {-2,0,+2} are exact in fp8e4; activations use a hi/lo split
  (a ~ q8(a) + q8(a - q8(a))) accumulated into the same PSUM group, which
  restores bf16-level accuracy at 2x bf16 speed (verified vs the reference:
  norm-rel ~9e-4 in simulation).
- ternarize: t = Sign(w/delta - 1) + Sign(w/delta + 1) in {-2, 0, +2} fp8.
  The 2x scale is washed out by the following BatchNorm.  delta = 0.7*mean|W|
  via per-core partial |.| row-sums + one tiny AllReduce + a ones-matmul
  broadcast.  Biases are dropped entirely (BN subtracts the mean).
- Each core ternarizes 1/8 of each hidden/output weight matrix into a
  DoubleRow-tiled fp8 layout and AllGathers it in 2 chunks.  The pipeline is
  2 layers deep: tern(l+1) runs during layer l-1's matmuls (via the bg pump),
  AllGather(l+1) runs during layer l, so layer l+1 never waits and the
  CC queue stays clear for that layer's BN-stats AllReduces.
- Activations kept transposed (features on partitions, batch on free dim).
  BN stats are free-dim reductions fused into the PSUM drains, AllReduced in
  4 chunks of 8 m-tiles per layer so only the last chunk's latency is
  exposed.  Pre-BN activations live in a rotating 24-tile bf16 pool.
- BN apply: ACT scale/bias pass, DVE hardtanh clamp, Pool-engine cast to fp8
  (hi), DVE subtract for the residual (lo).
- log-softmax: exp on ACT, partition-sum and broadcast via ones-matmuls on
  the PE (no DRAM round-trip), subtract, DMA out.
"""

import os

import numpy as np

N_CORES = 8
B = 8192
B_LOC = B // N_CORES          # 1024 rows per core
HID = 4096
N_MID = 4
KIN = 784
KIN_PAD = 1024                # 8 * 128 (even k-tiles for DoubleRow pairs)
KT_IN = KIN_PAD // 128        # 8
KP_IN = KT_IN // 2            # 4 k-pairs
KT_H = HID // 128             # 32
KP_H = KT_H // 2              # 16 k-pairs
MT = HID // 128               # 32 output-feature tiles
KL = 4                        # k-tiles per core shard of a hidden layer
SH_H = 512                    # hidden-weight rows per core (4096/8)
EPS = 1e-5
RG = [list(range(N_CORES))]
CH = 8                        # m-tiles per BN-stats chunk
NCH = MT // CH                # 4 chunks per layer
HRAW_POOL = 16                # rotating pre-BN bf16 tiles (2 chunks)

_cache = {}


def _build():
    n_mid_eff = int(os.environ.get("KERNEL_NMID", str(N_MID)))
    import concourse.bass as bass
    import concourse.bacc as bacc
    import concourse.mybir as mybir
    import concourse.tile as tile

    f32 = mybir.dt.float32
    bf16 = mybir.dt.bfloat16
    f8 = mybir.dt.float8e4
    AX = mybir.AxisListType
    OP = mybir.AluOpType
    AF = mybir.ActivationFunctionType
    PM = mybir.MatmulPerfMode.DoubleRow

    nc = bacc.Bacc("TRN2", target_bir_lowering=False, debug=False,
                   num_devices=N_CORES)

    # ---- I/O ----------------------------------------------------------------
    xT = nc.dram_tensor("xT", [KIN_PAD, B_LOC], f32, kind="ExternalInput")
    winT = nc.dram_tensor("winT", [KIN_PAD, HID], f32, kind="ExternalInput")
    whT = nc.dram_tensor("whT", [N_MID, SH_H, HID], f32, kind="ExternalInput")
    woT = nc.dram_tensor("woT", [SH_H, 16], f32, kind="ExternalInput")
    gam = nc.dram_tensor("gam", [N_MID + 1, HID], f32, kind="ExternalInput")
    bet = nc.dram_tensor("bet", [N_MID + 1, HID], f32, kind="ExternalInput")
    gob = nc.dram_tensor("gob", [2, 10], f32, kind="ExternalInput")
    # [10, B_LOC] layout: transposed on host (interleaved-partition DRAM
    # stores kill the device).
    out = nc.dram_tensor("out", [10, B_LOC], f32, kind="ExternalOutput")

    with tile.TileContext(nc) as tc:
        with (
            tc.tile_pool(name="ht", bufs=1) as ht,
            tc.tile_pool(name="hrawp", bufs=1) as hrawp,
            tc.tile_pool(name="wmp", bufs=2) as wmp,
            tc.tile_pool(name="slab", bufs=2) as slabp,
            tc.tile_pool(name="tern", bufs=2) as ternp,
            tc.tile_pool(name="small", bufs=1) as small,
            tc.tile_pool(name="stats", bufs=2) as stats,
            tc.tile_pool(name="psum", bufs=2, space="PSUM") as psum,
            tc.tile_pool(name="dram", bufs=1, space="DRAM") as dram,
        ):
            # ---- persistent small tiles ------------------------------------
            ones128 = small.tile([128, 128], f32, tag="ones128")
            nc.vector.memset(ones128, 1.0)
            ones10 = small.tile([10, 1], f32, tag="ones10")
            nc.vector.memset(ones10, 1.0)
            ones1x10 = small.tile([1, 10], f32, tag="ones1x10")
            nc.vector.memset(ones1x10, 1.0)
            negone = small.tile([128, 1], f32, tag="negone")
            nc.vector.memset(negone, -1.0)
            posone = small.tile([128, 1], f32, tag="posone")
            nc.vector.memset(posone, 1.0)
            epsb = small.tile([128, 1], f32, tag="epsb")
            nc.vector.memset(epsb, EPS)

            gam_sb, bet_sb = [], []
            for l in range(N_MID + 1):
                g = small.tile([128, MT], f32, tag=f"gam{l}", name=f"gam_sb{l}")
                nc.gpsimd.dma_start(out=g, in_=gam[l].rearrange("(m p) -> p m", p=128))
                gam_sb.append(g)
                b = small.tile([128, MT], f32, tag=f"bet{l}", name=f"bet_sb{l}")
                nc.gpsimd.dma_start(out=b, in_=bet[l].rearrange("(m p) -> p m", p=128))
                bet_sb.append(b)
            go_sb = small.tile([10, 1], f32, tag="go")
            nc.gpsimd.dma_start(out=go_sb, in_=gob[0:1, :].rearrange("a f -> f a"))
            bo_sb = small.tile([10, 1], f32, tag="bo")
            nc.gpsimd.dma_start(out=bo_sb, in_=gob[1:2, :].rearrange("a f -> f a"))

            # ---- activation double buffers (fp8 hi/lo k-pair tiles) --------
            # L0/H1/H3 write the A set; H0/H2 write the B set.  x aliases the
            # B set (its first writer is H0's norm pass, which runs after L0
            # has fully consumed x).
            AH = [ht.tile([128, 2, B_LOC], f8, tag=f"AH{p}", name=f"AH{p}")
                  for p in range(KP_H)]
            AL = [ht.tile([128, 2, B_LOC], f8, tag=f"AL{p}", name=f"AL{p}")
                  for p in range(KP_H)]
            BH = [ht.tile([128, 2, B_LOC], f8, tag=f"BH{p}", name=f"BH{p}")
                  for p in range(KP_H)]
            BL = [ht.tile([128, 2, B_LOC], f8, tag=f"BL{p}", name=f"BL{p}")
                  for p in range(KP_H)]
            XH = BH[:KP_IN]
            XL = BL[:KP_IN]

            # pre-BN bf16 scratch, rotating pool
            hraw = [hrawp.tile([128, B_LOC], bf16, tag=f"hr{j}", name=f"hr{j}")
                    for j in range(HRAW_POOL)]
            hraw_ctr = [0]

            # ---- DRAM scratch ----------------------------------------------
            tw_in = dram.tile([MT, KP_IN, 2, 128, 128], f8)
            tw_hid_sh = dram.tile([N_MID, MT, 2, 2, 128, 128], f8)
            # gathered halves: [rank, m-chunk 16, klp 2, two 2, 128, 128]
            tw_hid = [[dram.tile([N_CORES, MT // 2, 2, 2, 128, 128], f8,
                                 addr_space="Shared", tag=f"tw_hid{l}_{h}",
                                 name=f"tw_hid{l}_{h}")
                       for h in range(2)] for l in range(N_MID)]
            tw_out_sh = dram.tile([2, 2, 128, 16], f8)
            tw_out = dram.tile([N_CORES, 2, 2, 128, 16], f8,
                               addr_space="Shared")
            dlA_in = dram.tile([128, 1], f32)
            dlA_out = dram.tile([128, 1], f32, addr_space="Shared")
            dlB_in = dram.tile([128, 4], f32)
            dlB_out = dram.tile([128, 4], f32, addr_space="Shared")

            # ---- helpers ----------------------------------------------------
            def bcast_delta(partial_col, n_elems, nm):
                """[128,1] per-partition partial |W| sums -> broadcasted
                1/delta [128,1] (all partitions equal)."""
                ps = psum.tile([128, 1], f32, tag="small", name=f"dps_{nm}",
                               bufs=1)
                nc.tensor.matmul(ps, ones128, partial_col, start=True, stop=True)
                dsc = small.tile([128, 1], f32, tag=f"dsc_{nm}")
                nc.scalar.activation(out=dsc, in_=ps, func=AF.Copy,
                                     scale=0.7 / float(n_elems))
                inv = small.tile([128, 1], f32, tag=f"inv_{nm}")
                nc.vector.reciprocal(out=inv, in_=dsc)
                return inv

            def tern_slab(src_ap, inv_ap, dst_ap, cols):
                """ternarize one [128, cols] f32 slab -> {-2,0,2} fp8 in DRAM.
                dst_ap must be a [128, cols//128, 128] view."""
                sl = slabp.tile([128, cols], f32, tag="slab", name="tslab")
                nc.gpsimd.dma_start(out=sl, in_=src_ap)
                u = ternp.tile([128, cols], f8, tag="u", name="ternu")
                v = ternp.tile([128, cols], f8, tag="v", name="ternv")
                nc.scalar.activation(out=u, in_=sl, func=AF.Sign,
                                     bias=negone, scale=inv_ap)
                nc.scalar.activation(out=v, in_=sl, func=AF.Sign,
                                     bias=posone, scale=inv_ap)
                nc.vector.tensor_tensor(out=u, in0=u, in1=v, op=OP.add)
                nc.gpsimd.dma_start(out=dst_ap,
                                    in_=u.rearrange("p (m c) -> p m c", c=128))

            def delta_reduce(src_slabs, nm):
                """abs row-sum partials of a list of slab APs -> [128,1]."""
                part = small.tile([128, 32], f32, tag=f"part_{nm}")
                nc.vector.memset(part, 0.0)
                for s, (ap, cols) in enumerate(src_slabs):
                    sl = slabp.tile([128, cols], f32, tag="slab", name="dslab")
                    nc.gpsimd.dma_start(out=sl, in_=ap)
                    nc.vector.tensor_reduce(out=part[:, s:s + 1], in_=sl,
                                            axis=AX.X, op=OP.add,
                                            apply_absolute_value=True)
                tot = small.tile([128, 1], f32, tag=f"ptot_{nm}")
                nc.vector.tensor_reduce(out=tot, in_=part, axis=AX.X, op=OP.add)
                return tot

            # background work queue: thunks emitted interleaved into m-loops
            bg = []

            def pump(n=1):
                for _ in range(min(n, len(bg))):
                    bg.pop(0)()

            # ---- layer runner ----------------------------------------------
            def mm_layer(lname, hi_in, lo_in, n_kp, hi_out, lo_out,
                         w_read, wm_tag, gam_l, bet_l, pump_n=2):
                """One ternary-linear + BN + hardtanh layer, fp8 DoubleRow.

                hi_in/lo_in: [128, 2, B_LOC] fp8 k-pair tiles (input).
                hi_out/lo_out: same for the output.
                w_read(m, wm): fill wm [128, n_kp, 2, 128] fp8 for m-tile m.
                """
                S1 = stats.tile([128, MT], f32, tag="s1", name=f"S1_{lname}")
                S2 = stats.tile([128, MT], f32, tag="s2", name=f"S2_{lname}")
                sg = [None] * NCH

                def stats_chunk(c):
                    # AllReduce (sum, sumsq) for m-tiles [c*CH, (c+1)*CH)
                    bin_ = dram.tile([128, 2 * CH], f32,
                                     tag=f"bns_in_{lname}_{c}",
                                     name=f"bns_in_{lname}_{c}")
                    bout_ = dram.tile([128, 2 * CH], f32, addr_space="Shared",
                                      tag=f"bns_out_{lname}_{c}",
                                      name=f"bns_out_{lname}_{c}")
                    nc.gpsimd.dma_start(out=bin_[:, 0:CH],
                                        in_=S1[:, c * CH:(c + 1) * CH])
                    nc.gpsimd.dma_start(out=bin_[:, CH:2 * CH],
                                        in_=S2[:, c * CH:(c + 1) * CH])
                    nc.gpsimd.collective_compute(
                        "AllReduce", OP.add, replica_groups=RG,
                        ins=[bin_.opt()], outs=[bout_.opt()])
                    g = stats.tile([128, 2 * CH], f32, tag=f"sg{c}",
                                   name=f"sg_{lname}_{c}")
                    nc.gpsimd.dma_start(out=g, in_=bout_)
                    sg[c] = g

                def norm_chunk(c, raw_tiles):
                    # scale = gamma*rsqrt(var+eps); bias = beta - mean*scale
                    g = sg[c]
                    mean = stats.tile([128, CH], f32, tag="mean",
                                      name=f"mean_{lname}_{c}")
                    nc.vector.tensor_scalar_mul(mean, g[:, 0:CH], 1.0 / B)
                    ex2 = stats.tile([128, CH], f32, tag="ex2",
                                     name=f"ex2_{lname}_{c}")
                    nc.vector.tensor_scalar_mul(ex2, g[:, CH:2 * CH], 1.0 / B)
                    var = stats.tile([128, CH], f32, tag="var",
                                     name=f"var_{lname}_{c}")
                    nc.vector.tensor_tensor(out=var, in0=mean, in1=mean,
                                            op=OP.mult)
                    nc.vector.tensor_tensor(out=var, in0=ex2, in1=var,
                                            op=OP.subtract)
                    sd = stats.tile([128, CH], f32, tag="sd",
                                    name=f"sd_{lname}_{c}")
                    nc.scalar.activation(out=sd, in_=var, func=AF.Sqrt,
                                         bias=epsb)
                    rs = stats.tile([128, CH], f32, tag="rs",
                                    name=f"rs_{lname}_{c}")
                    nc.vector.reciprocal(out=rs, in_=sd)
                    scl = stats.tile([128, CH], f32, tag="scl",
                                     name=f"scl_{lname}_{c}")
                    nc.vector.tensor_tensor(out=scl, in0=rs,
                                            in1=gam_l[:, c * CH:(c + 1) * CH],
                                            op=OP.mult)
                    bia = stats.tile([128, CH], f32, tag="bia",
                                     name=f"bia_{lname}_{c}")
                    nc.vector.tensor_tensor(out=bia, in0=mean, in1=scl,
                                            op=OP.mult)
                    nc.vector.tensor_tensor(out=bia,
                                            in0=bet_l[:, c * CH:(c + 1) * CH],
                                            in1=bia, op=OP.subtract)
                    for j in range(CH):
                        m = c * CH + j
                        raw = raw_tiles[j]
                        # normalize (ACT), hardtanh clamp in place (DVE)
                        nc.scalar.activation(out=raw, in_=raw,
                                             func=AF.Identity,
                                             bias=bia[:, j:j + 1],
                                             scale=scl[:, j:j + 1])
                        nc.vector.tensor_scalar(
                            out=raw, in0=raw, scalar1=1.0, scalar2=-1.0,
                            op0=OP.min, op1=OP.max)
                        # hi = fp8(raw) on Pool; lo = raw - hi on DVE
                        hi_ap = hi_out[m // 2][:, m % 2, :]
                        nc.gpsimd.tensor_copy(out=hi_ap, in_=raw)
                        nc.vector.tensor_tensor(out=lo_out[m // 2][:, m % 2, :],
                                                in0=raw, in1=hi_ap,
                                                op=OP.subtract)

                chunk_raw = {}
                for m in range(MT):
                    wm = wmp.tile([128, n_kp, 2, 128], f8, tag=wm_tag,
                                  name=f"wm_{lname}_{m}")
                    w_read(m, wm)
                    ps = psum.tile([128, B_LOC], f32, tag="mm",
                                   name=f"ps_{lname}_{m}")
                    for kp in range(n_kp):
                        w_ap = wm[:, kp, :, :]
                        for hl, src in ((0, hi_in), (1, lo_in)):
                            for n in range(2):
                                nc.tensor.matmul(
                                    ps[:, n * 512:(n + 1) * 512],
                                    w_ap,
                                    src[kp][:, :, n * 512:(n + 1) * 512],
                                    start=(kp == 0 and hl == 0),
                                    stop=(kp == n_kp - 1 and hl == 1),
                                    perf_mode=PM)
                    raw = hraw[hraw_ctr[0] % HRAW_POOL]
                    hraw_ctr[0] += 1
                    chunk_raw.setdefault(m // CH, []).append(raw)
                    nc.vector.tensor_scalar(
                        out=raw, in0=ps, scalar1=1.0, scalar2=None,
                        op0=OP.mult, op1=OP.add, accum_out=S1[:, m:m + 1])
                    sj = ternp.tile([128, B_LOC], bf16, tag="sq", name="sqj")
                    nc.scalar.activation(out=sj, in_=ps, func=AF.Square,
                                         accum_out=S2[:, m:m + 1])
                    if m % CH == CH - 1:
                        c = m // CH
                        stats_chunk(c)
                        norm_chunk(c, chunk_raw.pop(c))
                    pump(pump_n)
                pump(len(bg))

            # ================= startup ======================================
            # delta partials for hid0 shard first (gates H0's AllGather)
            wv_h = [whT[l].rearrange("(kl p) f -> kl p f", p=128)
                    for l in range(N_MID)]
            if n_mid_eff > 0:
                h0_slabs = [(wv_h[0][kl][:, h * 1024:(h + 1) * 1024], 1024)
                            for kl in range(KL) for h in range(4)]
                ph0 = delta_reduce(h0_slabs, "h0")
                nc.gpsimd.dma_start(out=dlA_in, in_=ph0)
                nc.gpsimd.collective_compute(
                    "AllReduce", OP.add, replica_groups=RG,
                    ins=[dlA_in.opt()], outs=[dlA_out.opt()])

            # x load + hi/lo cast (feeds input-layer matmuls)
            xv = xT.rearrange("(t p) b -> t p b", p=128)
            for t in range(KT_IN):
                xs = slabp.tile([128, B_LOC], f32, tag="slab", name=f"xs{t}")
                nc.sync.dma_start(out=xs, in_=xv[t])
                hi_ap = XH[t // 2][:, t % 2, :]
                nc.gpsimd.tensor_copy(out=hi_ap, in_=xs)
                nc.vector.tensor_tensor(out=XL[t // 2][:, t % 2, :],
                                        in0=xs, in1=hi_ap, op=OP.subtract)

            # delta for W_in (local, full matrix on every core)
            wv_in = winT.rearrange("(t p) f -> t p f", p=128)
            in_slabs = [(wv_in[t][:, h * 1024:(h + 1) * 1024], 1024)
                        for t in range(KT_IN) for h in range(4)]
            pin = delta_reduce(in_slabs, "in")
            inv_in = bcast_delta(pin, KIN * HID, "in")

            if n_mid_eff > 0:
                ph0g = small.tile([128, 1], f32, tag="ph0g")
                nc.gpsimd.dma_start(out=ph0g, in_=dlA_out)
                inv_h0 = bcast_delta(ph0g, HID * HID, "h0")
                # ternarize hid0 shard + AllGather halves (runs during L0)
                for kl in range(KL):
                    for h in range(4):
                        tern_slab(
                            wv_h[0][kl][:, h * 1024:(h + 1) * 1024], inv_h0,
                            tw_hid_sh[0, h * 8:(h + 1) * 8, kl // 2,
                                      kl % 2].rearrange("m p c -> p m c"),
                            1024)
                for h in range(2):
                    nc.gpsimd.collective_compute(
                        "AllGather", OP.bypass, replica_groups=RG,
                        ins=[tw_hid_sh[0, h * 16:(h + 1) * 16].opt()],
                        outs=[tw_hid[0][h].opt()])

            # ternarize W_in, h-major so L0's m-tiles become ready in order
            for h in range(4):
                for t in range(KT_IN):
                    tern_slab(wv_in[t][:, h * 1024:(h + 1) * 1024], inv_in,
                              tw_in[h * 8:(h + 1) * 8, t // 2,
                                    t % 2].rearrange("m p c -> p m c"),
                              1024)

            # delta partials for hid1..3 + out -> AllReduce #2.
            # Split into per-slab bg thunks so L0's own DMA stays interleaved.
            invs = {}
            dparts = {}

            def mk_dpart_thunk(l, idx, ap, cols):
                def t():
                    if l not in dparts:
                        dparts[l] = small.tile([128, 16], f32,
                                               tag=f"part_h{l}",
                                               name=f"part_h{l}")
                        nc.vector.memset(dparts[l], 0.0)
                    sl = slabp.tile([128, cols], f32, tag="slab", name="dslab")
                    nc.gpsimd.dma_start(out=sl, in_=ap)
                    nc.vector.tensor_reduce(out=dparts[l][:, idx:idx + 1],
                                            in_=sl, axis=AX.X, op=OP.add,
                                            apply_absolute_value=True)
                return t

            def emit_delta_rest_final():
                pb = small.tile([128, 4], f32, tag="pb")
                nc.vector.memset(pb, 0.0)
                for i, l in enumerate(range(1, n_mid_eff)):
                    nc.vector.tensor_reduce(out=pb[:, i:i + 1], in_=dparts[l],
                                            axis=AX.X, op=OP.add)
                wv_o = woT.rearrange("(s p) c -> s p c", p=128)
                o_slabs = [(wv_o[s], 16) for s in range(4)]
                po = delta_reduce(o_slabs, "out")
                nc.vector.tensor_copy(out=pb[:, 3:4], in_=po)
                nc.gpsimd.dma_start(out=dlB_in, in_=pb)
                nc.gpsimd.collective_compute(
                    "AllReduce", OP.add, replica_groups=RG,
                    ins=[dlB_in.opt()], outs=[dlB_out.opt()])
                pbg = small.tile([128, 4], f32, tag="pbg")
                nc.gpsimd.dma_start(out=pbg, in_=dlB_out)
                for i, l in enumerate(range(1, n_mid_eff)):
                    invs[l] = bcast_delta(pbg[:, i:i + 1], HID * HID, f"h{l}")
                invs["out"] = bcast_delta(pbg[:, 3:4], 10 * HID, "out")

            for l in range(1, n_mid_eff):
                for idx, (kl, h) in enumerate(
                        (kl, h) for kl in range(KL) for h in range(4)):
                    bg.append(mk_dpart_thunk(
                        l, idx, wv_h[l][kl][:, h * 1024:(h + 1) * 1024], 1024))
            bg.append(emit_delta_rest_final)

            def emit_tern_hid(l):
                # invs[l] is looked up lazily: the delta thunks run first
                for kl in range(KL):
                    for h in range(4):
                        bg.append(lambda l=l, kl=kl, h=h: tern_slab(
                            wv_h[l][kl][:, h * 1024:(h + 1) * 1024], invs[l],
                            tw_hid_sh[l, h * 8:(h + 1) * 8, kl // 2,
                                      kl % 2].rearrange("m p c -> p m c"),
                            1024))

            def emit_ag_hid(l):
                for h in range(2):
                    nc.gpsimd.collective_compute(
                        "AllGather", OP.bypass, replica_groups=RG,
                        ins=[tw_hid_sh[l, h * 16:(h + 1) * 16].opt()],
                        outs=[tw_hid[l][h].opt()])

            def emit_tern_out():
                wv_o2 = woT.rearrange("(s p) c -> s p c", p=128)
                for s in range(4):
                    sl = slabp.tile([128, 16], f32, tag="oslab",
                                    name="oslab")
                    nc.gpsimd.dma_start(out=sl, in_=wv_o2[s])
                    u = ternp.tile([128, 16], f8, tag="ou", name="ou")
                    v = ternp.tile([128, 16], f8, tag="ov", name="ov")
                    nc.scalar.activation(out=u, in_=sl, func=AF.Sign,
                                         bias=negone, scale=invs["out"])
                    nc.scalar.activation(out=v, in_=sl, func=AF.Sign,
                                         bias=posone, scale=invs["out"])
                    nc.vector.tensor_tensor(out=u, in0=u, in1=v, op=OP.add)
                    nc.gpsimd.dma_start(out=tw_out_sh[s // 2, s % 2], in_=u)
                nc.gpsimd.collective_compute(
                    "AllGather", OP.bypass, replica_groups=RG,
                    ins=[tw_out_sh.opt()], outs=[tw_out.opt()])

            # ================= layers =======================================
            def w_read_in(m, wm):
                nc.sync.dma_start(
                    out=wm,
                    in_=tw_in[m].rearrange("kp two p c -> p kp two c"))

            def w_read_hid(l):
                def f(m, wm):
                    half = tw_hid[l][m // 16]
                    for r in range(N_CORES):
                        nc.sync.dma_start(
                            out=wm[:, r * 2:(r + 1) * 2, :, :],
                            in_=half[r, m % 16].rearrange(
                                "klp two p c -> p klp two c"))
                return f

            bufs = [(AH, AL), (BH, BL)]

            # tern(1) is pumped during L0; AG(l+1) is emitted right before
            # layer H{l} so it runs during H{l} and is done for H{l+1}.
            if n_mid_eff > 1:
                emit_tern_hid(1)
            if n_mid_eff == 0:
                bg.append(emit_tern_out)
            mm_layer("L0", XH, XL, KP_IN, AH, AL, w_read_in, "wmin",
                     gam_sb[0], bet_sb[0], pump_n=3)

            for l in range(n_mid_eff):
                if l + 1 < n_mid_eff:
                    emit_ag_hid(l + 1)
                    if l + 2 < n_mid_eff:
                        emit_tern_hid(l + 2)
                    else:
                        bg.append(emit_tern_out)
                hi_in, lo_in = bufs[l % 2]
                hi_out, lo_out = bufs[(l + 1) % 2]
                mm_layer(f"H{l}", hi_in, lo_in, KP_H, hi_out, lo_out,
                         w_read_hid(l), "wm", gam_sb[l + 1], bet_sb[l + 1])

            # ================= output layer + log-softmax ===================
            hi_fin, lo_fin = bufs[n_mid_eff % 2]
            wmo = wmp.tile([128, KP_H, 2, 16], f8, tag="wmo", name="wmo")
            nc.sync.dma_start(
                out=wmo,
                in_=tw_out.rearrange("r klp two p c -> p (r klp) two c"))
            pso = psum.tile([16, B_LOC], f32, tag="mm", name="pso")
            for kp in range(KP_H):
                w_ap = wmo[:, kp, :, :]
                for hl, src in ((0, hi_fin), (1, lo_fin)):
                    for n in range(2):
                        nc.tensor.matmul(
                            pso[:, n * 512:(n + 1) * 512],
                            w_ap,
                            src[kp][:, :, n * 512:(n + 1) * 512],
                            start=(kp == 0 and hl == 0),
                            stop=(kp == KP_H - 1 and hl == 1),
                            perf_mode=PM)
            S1o = stats.tile([10, 1], f32, tag="s1o")
            S2o = stats.tile([10, 1], f32, tag="s2o")
            opre = small.tile([10, B_LOC], f32, tag="opre")
            nc.vector.tensor_scalar(out=opre, in0=pso[0:10, :], scalar1=1.0,
                                    scalar2=None, op0=OP.mult, op1=OP.add,
                                    accum_out=S1o)
            sjo = ternp.tile([10, B_LOC], bf16, tag="sq", name="sqo")
            nc.scalar.activation(out=sjo, in_=pso[0:10, :], func=AF.Square,
                                 accum_out=S2o)
            bno_in = dram.tile([10, 2], f32)
            bno_out = dram.tile([10, 2], f32, addr_space="Shared")
            s12o = stats.tile([10, 2], f32, tag="s12o")
            nc.vector.tensor_copy(out=s12o[:, 0:1], in_=S1o)
            nc.vector.tensor_copy(out=s12o[:, 1:2], in_=S2o)
            nc.gpsimd.dma_start(out=bno_in, in_=s12o)
            nc.gpsimd.collective_compute(
                "AllReduce", OP.add, replica_groups=RG,
                ins=[bno_in.opt()], outs=[bno_out.opt()])
            sgo = stats.tile([10, 2], f32, tag="sgo")
            nc.gpsimd.dma_start(out=sgo, in_=bno_out)
            meano = stats.tile([10, 1], f32, tag="meano")
            nc.vector.tensor_scalar_mul(meano, sgo[:, 0:1], 1.0 / B)
            ex2o = stats.tile([10, 1], f32, tag="ex2o")
            nc.vector.tensor_scalar_mul(ex2o, sgo[:, 1:2], 1.0 / B)
            msqo = stats.tile([10, 1], f32, tag="msqo")
            nc.vector.tensor_tensor(out=msqo, in0=meano, in1=meano, op=OP.mult)
            varo = stats.tile([10, 1], f32, tag="varo")
            nc.vector.tensor_tensor(out=varo, in0=ex2o, in1=msqo,
                                    op=OP.subtract)
            sdo = stats.tile([10, 1], f32, tag="sdo")
            nc.scalar.activation(out=sdo, in_=varo, func=AF.Sqrt,
                                 bias=epsb[0:10, :])
            rso = stats.tile([10, 1], f32, tag="rso")
            nc.vector.reciprocal(out=rso, in_=sdo)
            sclo = stats.tile([10, 1], f32, tag="sclo")
            nc.vector.tensor_tensor(out=sclo, in0=rso, in1=go_sb, op=OP.mult)
            mso = stats.tile([10, 1], f32, tag="mso")
            nc.vector.tensor_tensor(out=mso, in0=meano, in1=sclo, op=OP.mult)
            biao = stats.tile([10, 1], f32, tag="biao")
            nc.vector.tensor_tensor(out=biao, in0=bo_sb, in1=mso,
                                    op=OP.subtract)
            onorm = opre
            nc.scalar.activation(out=onorm, in_=opre, func=AF.Identity,
                                 bias=biao, scale=sclo)
            esb = small.tile([10, B_LOC], f32, tag="esb")
            nc.scalar.activation(out=esb, in_=onorm, func=AF.Exp)
            csp = psum.tile([1, B_LOC], f32, tag="cs", bufs=1)
            for n in range(2):
                nc.tensor.matmul(csp[:, n * 512:(n + 1) * 512], ones10,
                                 esb[:, n * 512:(n + 1) * 512],
                                 start=True, stop=True)
            lsb = small.tile([1, B_LOC], f32, tag="lsb")
            nc.scalar.activation(out=lsb, in_=csp, func=AF.Ln)
            lse_bc = psum.tile([10, B_LOC], f32, tag="cs", name="lse_bc",
                               bufs=1)
            for n in range(2):
                nc.tensor.matmul(lse_bc[:, n * 512:(n + 1) * 512], ones1x10,
                                 lsb[:, n * 512:(n + 1) * 512],
                                 start=True, stop=True)
            fout = small.tile([10, B_LOC], f32, tag="esb", name="fout")
            nc.vector.tensor_tensor(out=fout, in0=onorm, in1=lse_bc,
                                    op=OP.subtract)
            nc.gpsimd.dma_start(out=out[:], in_=fout)

    nc.compile()
    return nc


def _get_program():
    if "nc" not in _cache:
        _cache["nc"] = _build()
    return _cache["nc"]


def kernel(x, W_in, b_in, W_hid, b_hid, W_out, b_out, gamma, beta,
           gamma_out, beta_out):
    from concourse.bass_utils import run_bass_kernel_spmd

    nc = _get_program()

    x = np.asarray(x, dtype=np.float32).reshape(B, KIN)
    # layout-only host prep (transpose + zero-pad + shard)
    xT_full = np.zeros((KIN_PAD, B), dtype=np.float32)
    xT_full[:KIN] = x.T
    winT_full = np.zeros((KIN_PAD, HID), dtype=np.float32)
    winT_full[:KIN] = np.asarray(W_in, dtype=np.float32).T
    whT_full = np.ascontiguousarray(
        np.asarray(W_hid, dtype=np.float32).transpose(0, 2, 1))
    woT_full = np.zeros((HID, 16), dtype=np.float32)
    woT_full[:, :10] = np.asarray(W_out, dtype=np.float32).T
    gam_np = np.ascontiguousarray(np.asarray(gamma, dtype=np.float32))
    bet_np = np.ascontiguousarray(np.asarray(beta, dtype=np.float32))
    gob_np = np.stack([np.asarray(gamma_out, dtype=np.float32),
                       np.asarray(beta_out, dtype=np.float32)])

    in_maps = []
    for c in range(N_CORES):
        in_maps.append({
            "xT": np.ascontiguousarray(
                xT_full[:, c * B_LOC:(c + 1) * B_LOC]),
            "winT": winT_full,
            "whT": np.ascontiguousarray(
                whT_full[:, c * SH_H:(c + 1) * SH_H, :]),
            "woT": np.ascontiguousarray(
                woT_full[c * SH_H:(c + 1) * SH_H, :]),
            "gam": gam_np,
            "bet": bet_np,
            "gob": gob_np,
        })

    res = run_bass_kernel_spmd(nc, in_maps, core_ids=list(range(N_CORES)))
    return np.concatenate(
        [np.ascontiguousarray(res.results[c]["out"].T) for c in range(N_CORES)],
        axis=0)


# revision 15
# speedup vs baseline: 1.2385x; 1.2290x over previous
"""Trainium2 Bass kernel for nn_MnistNet (ternary-weight MLP with training-mode
BatchNorm), data-parallel over batch across 8 NeuronCores.

Strategy (v3: fp8 DoubleRow, multi-queue DMA, partial hi/lo)
------------------------------------------------------------
- Host side does layout-only prep: transpose x / weights, zero-pad 784->1024,
  shard the batch (1024 rows/core) and the weight rows (1/8 per core).
- All matmuls run in fp8e4 with MatmulPerfMode.DoubleRow (2 k-tiles per
  instruction; measured on hw: same 262ns cadence as a bf16 matmul, so 2x
  throughput per instruction).  Ternary weights {-2,0,+2} are exact in fp8e4.
- Activations use a hi/lo split (a ~ q8(a) + q8(a - q8(a))) accumulated into
  the same PSUM group where accuracy requires it; layers H2/H3 run plain fp8
  (hi only), which halves their matmul count.  Simulated end-to-end norm-rel
  8.9e-3 (hw-calibrated ~1.0e-2) vs the 2e-2 gate.
- ternarize: t = Sign(w/delta - 1) + Sign(w/delta + 1) in {-2, 0, +2} fp8.
  The 2x scale is washed out by the following BatchNorm.  delta = 0.7*mean|W|
  via per-core partial |.| row-sums + one tiny AllReduce + a ones-matmul
  broadcast.  Biases are dropped entirely (BN subtracts the mean).
- Startup DMA (delta + ternarize passes over the f32 weights) is issued as
  [128, 2048] slabs round-robined over the sync/tensor/gpsimd queues: a
  single queue sustains only ~112 GB/s, so concurrency across queues is
  what approaches the HBM roofline.  The all-zero padding k-tile of W_in is
  never read (memset + store instead).
- Each core ternarizes 1/8 of each hidden/output weight matrix into a
  DoubleRow-tiled fp8 layout and AllGathers it in 2 chunks.  The pipeline is
  2 layers deep: tern(l+1) runs during layer l-1's matmuls (via the bg pump),
  AllGather(l+1) runs during layer l, so layer l+1 never waits and the CC
  queue stays clear for the BN-stats AllReduces.
- Activations kept transposed (features on partitions, batch on free dim).
  BN stats are free-dim reductions fused into the PSUM drains, AllReduced in
  8 chunks of 4 m-tiles per layer so only the last chunk's latency is
  exposed.  Pre-BN activations live in a rotating 12-tile bf16 pool; the
  sum-of-squares pass reads those bf16 tiles (2x DVE rate vs f32 PSUM).
- BN apply: ACT scale/bias pass, DVE hardtanh clamp, Pool-engine cast to fp8
  (hi), DVE subtract for the residual (lo, only where consumed).
- log-softmax: exp on ACT, partition-sum and broadcast via ones-matmuls on
  the PE (no DRAM round-trip), subtract, DMA out.
"""

import os

import numpy as np

N_CORES = 8
B = 8192
B_LOC = B // N_CORES          # 1024 rows per core
HID = 4096
N_MID = 4
KIN = 784
KIN_PAD = 1024                # 8 * 128 (even k-tiles for DoubleRow pairs)
KT_IN = KIN_PAD // 128        # 8
KT_IN_NZ = 7                  # k-tiles with any nonzero data (784 < 7*128)
KP_IN = KT_IN // 2            # 4 k-pairs
KT_H = HID // 128             # 32
KP_H = KT_H // 2              # 16 k-pairs
MT = HID // 128               # 32 output-feature tiles
KL = 4                        # k-tiles per core shard of a hidden layer
SH_H = 512                    # hidden-weight rows per core (4096/8)
EPS = 1e-5
RG = [list(range(N_CORES))]
CH = 4                        # m-tiles per BN-stats chunk
NCH = MT // CH                # 8 chunks per layer
HRAW_POOL = 12                # rotating pre-BN bf16 tiles (3 chunks)
# fp8 hi/lo schedule: layers H2/H3 run plain fp8 (their inputs skip the lo
# residual matmuls); everything else keeps the hi/lo split.
USE_LO_IN = {"L0": True, "H0": True, "H1": True, "H2": False, "H3": False,
             "out": True}

_cache = {}


def _build():
    n_mid_eff = int(os.environ.get("KERNEL_NMID", str(N_MID)))
    import concourse.bass as bass
    import concourse.bacc as bacc
    import concourse.mybir as mybir
    import concourse.tile as tile

    f32 = mybir.dt.float32
    bf16 = mybir.dt.bfloat16
    f8 = mybir.dt.float8e4
    AX = mybir.AxisListType
    OP = mybir.AluOpType
    AF = mybir.ActivationFunctionType
    PM = mybir.MatmulPerfMode.DoubleRow

    nc = bacc.Bacc("TRN2", target_bir_lowering=False, debug=False,
                   num_devices=N_CORES)

    def use_lo_in(lname):
        return USE_LO_IN.get(lname, True) and n_mid_eff == N_MID

    # ---- I/O ----------------------------------------------------------------
    xT = nc.dram_tensor("xT", [KIN_PAD, B_LOC], f32, kind="ExternalInput")
    winT = nc.dram_tensor("winT", [KIN_PAD, HID], f32, kind="ExternalInput")
    whT = nc.dram_tensor("whT", [N_MID, SH_H, HID], f32, kind="ExternalInput")
    woT = nc.dram_tensor("woT", [SH_H, 16], f32, kind="ExternalInput")
    gam = nc.dram_tensor("gam", [N_MID + 1, HID], f32, kind="ExternalInput")
    bet = nc.dram_tensor("bet", [N_MID + 1, HID], f32, kind="ExternalInput")
    gob = nc.dram_tensor("gob", [2, 10], f32, kind="ExternalInput")
    # [10, B_LOC] layout: transposed on host (interleaved-partition DRAM
    # stores kill the device).
    out = nc.dram_tensor("out", [10, B_LOC], f32, kind="ExternalOutput")

    with tile.TileContext(nc) as tc:
        with (
            tc.tile_pool(name="ht", bufs=1) as ht,
            tc.tile_pool(name="hrawp", bufs=1) as hrawp,
            tc.tile_pool(name="wmp", bufs=2) as wmp,
            tc.tile_pool(name="slab", bufs=2) as slabp,
            tc.tile_pool(name="tern", bufs=2) as ternp,
            tc.tile_pool(name="small", bufs=1) as small,
            tc.tile_pool(name="stats", bufs=2) as stats,
            tc.tile_pool(name="psum", bufs=2, space="PSUM") as psum,
            tc.tile_pool(name="dram", bufs=1, space="DRAM") as dram,
        ):
            # DMA queue rotation: only sync/scalar/gpsimd can initiate DMAs.
            # Delta-pass loads may use the scalar queue (idle then); the
            # ternarize loads avoid it (it runs the Sign ops).
            qstate = {"delta": [nc.sync, nc.scalar, nc.gpsimd],
                      "tern": [nc.sync, nc.gpsimd], "i": 0}

            def dq(kind="delta"):
                qstate["i"] += 1
                qs = qstate[kind]
                return qs[qstate["i"] % len(qs)]

            # ---- persistent small tiles ------------------------------------
            ones128 = small.tile([128, 128], f32, tag="ones128")
            nc.vector.memset(ones128, 1.0)
            ones10 = small.tile([10, 1], f32, tag="ones10")
            nc.vector.memset(ones10, 1.0)
            ones1x10 = small.tile([1, 10], f32, tag="ones1x10")
            nc.vector.memset(ones1x10, 1.0)
            negone = small.tile([128, 1], f32, tag="negone")
            nc.vector.memset(negone, -1.0)
            posone = small.tile([128, 1], f32, tag="posone")
            nc.vector.memset(posone, 1.0)
            epsb = small.tile([128, 1], f32, tag="epsb")
            nc.vector.memset(epsb, EPS)

            gam_sb, bet_sb = [], []
            for l in range(N_MID + 1):
                g = small.tile([128, MT], f32, tag=f"gam{l}", name=f"gam_sb{l}")
                nc.gpsimd.dma_start(out=g, in_=gam[l].rearrange("(m p) -> p m", p=128))
                gam_sb.append(g)
                b = small.tile([128, MT], f32, tag=f"bet{l}", name=f"bet_sb{l}")
                nc.gpsimd.dma_start(out=b, in_=bet[l].rearrange("(m p) -> p m", p=128))
                bet_sb.append(b)
            go_sb = small.tile([10, 1], f32, tag="go")
            nc.gpsimd.dma_start(out=go_sb, in_=gob[0:1, :].rearrange("a f -> f a"))
            bo_sb = small.tile([10, 1], f32, tag="bo")
            nc.gpsimd.dma_start(out=bo_sb, in_=gob[1:2, :].rearrange("a f -> f a"))

            # ---- activation double buffers (fp8 hi/lo k-pair tiles) --------
            # L0/H1/H3 write the A set; H0/H2 write the B set.  x aliases the
            # B set (its first writer is H0's norm pass, which runs after L0
            # has fully consumed x).
            AH = [ht.tile([128, 2, B_LOC], f8, tag=f"AH{p}", name=f"AH{p}")
                  for p in range(KP_H)]
            AL = [ht.tile([128, 2, B_LOC], f8, tag=f"AL{p}", name=f"AL{p}")
                  for p in range(KP_H)]
            BH = [ht.tile([128, 2, B_LOC], f8, tag=f"BH{p}", name=f"BH{p}")
                  for p in range(KP_H)]
            BL = [ht.tile([128, 2, B_LOC], f8, tag=f"BL{p}", name=f"BL{p}")
                  for p in range(KP_H)]
            XH = BH[:KP_IN]
            XL = BL[:KP_IN]

            # pre-BN bf16 scratch, rotating pool
            hraw = [hrawp.tile([128, B_LOC], bf16, tag=f"hr{j}", name=f"hr{j}")
                    for j in range(HRAW_POOL)]
            hraw_ctr = [0]

            # ---- DRAM scratch ----------------------------------------------
            tw_in = dram.tile([MT, KP_IN, 2, 128, 128], f8)
            tw_hid_sh = dram.tile([N_MID, MT, 2, 2, 128, 128], f8)
            # gathered halves: [rank, m-chunk 16, klp 2, two 2, 128, 128]
            tw_hid = [[dram.tile([N_CORES, MT // 2, 2, 2, 128, 128], f8,
                                 addr_space="Shared", tag=f"tw_hid{l}_{h}",
                                 name=f"tw_hid{l}_{h}")
                       for h in range(2)] for l in range(N_MID)]
            tw_out_sh = dram.tile([2, 2, 128, 16], f8)
            tw_out = dram.tile([N_CORES, 2, 2, 128, 16], f8,
                               addr_space="Shared")
            dlA_in = dram.tile([128, 1], f32)
            dlA_out = dram.tile([128, 1], f32, addr_space="Shared")
            dlB_in = dram.tile([128, 4], f32)
            dlB_out = dram.tile([128, 4], f32, addr_space="Shared")

            # ---- helpers ----------------------------------------------------
            def bcast_delta(partial_col, n_elems, nm):
                """[128,1] per-partition partial |W| sums -> broadcasted
                1/delta [128,1] (all partitions equal)."""
                ps = psum.tile([128, 1], f32, tag="small", name=f"dps_{nm}",
                               bufs=1)
                nc.tensor.matmul(ps, ones128, partial_col, start=True, stop=True)
                dsc = small.tile([128, 1], f32, tag=f"dsc_{nm}")
                nc.scalar.activation(out=dsc, in_=ps, func=AF.Copy,
                                     scale=0.7 / float(n_elems))
                inv = small.tile([128, 1], f32, tag=f"inv_{nm}")
                nc.vector.reciprocal(out=inv, in_=dsc)
                return inv

            def tern_slab(src_ap, inv_ap, dst_ap, cols):
                """ternarize one [128, cols] f32 slab -> {-2,0,2} fp8 in DRAM.
                dst_ap must be a [128, cols//128, 128] view."""
                sl = slabp.tile([128, cols], f32, tag="slab", name="tslab")
                dq("tern").dma_start(out=sl, in_=src_ap)
                u = ternp.tile([128, cols], f8, tag="u", name="ternu")
                v = ternp.tile([128, cols], f8, tag="v", name="ternv")
                nc.scalar.activation(out=u, in_=sl, func=AF.Sign,
                                     bias=negone, scale=inv_ap)
                nc.scalar.activation(out=v, in_=sl, func=AF.Sign,
                                     bias=posone, scale=inv_ap)
                nc.vector.tensor_tensor(out=u, in0=u, in1=v, op=OP.add)
                nc.gpsimd.dma_start(out=dst_ap,
                                    in_=u.rearrange("p (m c) -> p m c", c=128))

            def delta_reduce(src_slabs, nm):
                """abs row-sum partials of a list of slab APs -> [128,1]."""
                part = small.tile([128, 16], f32, tag=f"part_{nm}",
                                  name=f"part_{nm}")
                nc.vector.memset(part, 0.0)
                for s, (ap, cols) in enumerate(src_slabs):
                    sl = slabp.tile([128, cols], f32, tag="slab", name="dslab")
                    dq().dma_start(out=sl, in_=ap)
                    nc.vector.tensor_reduce(out=part[:, s:s + 1], in_=sl,
                                            axis=AX.X, op=OP.add,
                                            apply_absolute_value=True)
                tot = small.tile([128, 1], f32, tag=f"ptot_{nm}",
                                 name=f"ptot_{nm}")
                nc.vector.tensor_reduce(out=tot, in_=part, axis=AX.X, op=OP.add)
                return tot

            # background work queue: thunks emitted interleaved into m-loops
            bg = []

            def pump(n=1):
                for _ in range(min(n, len(bg))):
                    bg.pop(0)()

            # ---- layer runner ----------------------------------------------
            def mm_layer(lname, hi_in, lo_in, n_kp, hi_out, lo_out,
                         w_read, wm_tag, gam_l, bet_l, make_lo, pump_n=2):
                """One ternary-linear + BN + hardtanh layer, fp8 DoubleRow.

                hi_in/lo_in: [128, 2, B_LOC] fp8 k-pair tiles (input).
                hi_out/lo_out: same for the output (lo skipped if not make_lo).
                w_read(m, wm): fill wm [128, n_kp, 2, 128] fp8 for m-tile m.
                """
                srcs = [(0, hi_in)]
                if use_lo_in(lname):
                    srcs.append((1, lo_in))
                last_hl = srcs[-1][0]
                S1 = stats.tile([128, MT], f32, tag="s1", name=f"S1_{lname}")
                S2 = stats.tile([128, MT], f32, tag="s2", name=f"S2_{lname}")
                sg = [None] * NCH

                def stats_chunk(c):
                    # AllReduce (sum, sumsq) for m-tiles [c*CH, (c+1)*CH)
                    bin_ = dram.tile([128, 2 * CH], f32,
                                     tag=f"bns_in_{lname}_{c}",
                                     name=f"bns_in_{lname}_{c}")
                    bout_ = dram.tile([128, 2 * CH], f32, addr_space="Shared",
                                      tag=f"bns_out_{lname}_{c}",
                                      name=f"bns_out_{lname}_{c}")
                    nc.gpsimd.dma_start(out=bin_[:, 0:CH],
                                        in_=S1[:, c * CH:(c + 1) * CH])
                    nc.gpsimd.dma_start(out=bin_[:, CH:2 * CH],
                                        in_=S2[:, c * CH:(c + 1) * CH])
                    nc.gpsimd.collective_compute(
                        "AllReduce", OP.add, replica_groups=RG,
                        ins=[bin_.opt()], outs=[bout_.opt()])
                    g = stats.tile([128, 2 * CH], f32, tag=f"sg{c}",
                                   name=f"sg_{lname}_{c}")
                    nc.gpsimd.dma_start(out=g, in_=bout_)
                    sg[c] = g

                def norm_chunk(c, raw_tiles):
                    # scale = gamma*rsqrt(var+eps); bias = beta - mean*scale
                    g = sg[c]
                    mean = stats.tile([128, CH], f32, tag="mean",
                                      name=f"mean_{lname}_{c}")
                    nc.vector.tensor_scalar_mul(mean, g[:, 0:CH], 1.0 / B)
                    ex2 = stats.tile([128, CH], f32, tag="ex2",
                                     name=f"ex2_{lname}_{c}")
                    nc.vector.tensor_scalar_mul(ex2, g[:, CH:2 * CH], 1.0 / B)
                    var = stats.tile([128, CH], f32, tag="var",
                                     name=f"var_{lname}_{c}")
                    nc.vector.tensor_tensor(out=var, in0=mean, in1=mean,
                                            op=OP.mult)
                    nc.vector.tensor_tensor(out=var, in0=ex2, in1=var,
                                            op=OP.subtract)
                    sd = stats.tile([128, CH], f32, tag="sd",
                                    name=f"sd_{lname}_{c}")
                    nc.scalar.activation(out=sd, in_=var, func=AF.Sqrt,
                                         bias=epsb)
                    rs = stats.tile([128, CH], f32, tag="rs",
                                    name=f"rs_{lname}_{c}")
                    nc.vector.reciprocal(out=rs, in_=sd)
                    scl = stats.tile([128, CH], f32, tag="scl",
                                     name=f"scl_{lname}_{c}")
                    nc.vector.tensor_tensor(out=scl, in0=rs,
                                            in1=gam_l[:, c * CH:(c + 1) * CH],
                                            op=OP.mult)
                    bia = stats.tile([128, CH], f32, tag="bia",
                                     name=f"bia_{lname}_{c}")
                    nc.vector.tensor_tensor(out=bia, in0=mean, in1=scl,
                                            op=OP.mult)
                    nc.vector.tensor_tensor(out=bia,
                                            in0=bet_l[:, c * CH:(c + 1) * CH],
                                            in1=bia, op=OP.subtract)
                    for j in range(CH):
                        m = c * CH + j
                        raw = raw_tiles[j]
                        # normalize (ACT), hardtanh clamp in place (DVE)
                        nc.scalar.activation(out=raw, in_=raw,
                                             func=AF.Identity,
                                             bias=bia[:, j:j + 1],
                                             scale=scl[:, j:j + 1])
                        nc.vector.tensor_scalar(
                            out=raw, in0=raw, scalar1=1.0, scalar2=-1.0,
                            op0=OP.min, op1=OP.max)
                        # hi = fp8(raw) on Pool; lo = raw - hi on DVE
                        hi_ap = hi_out[m // 2][:, m % 2, :]
                        nc.gpsimd.tensor_copy(out=hi_ap, in_=raw)
                        if make_lo:
                            nc.vector.tensor_tensor(
                                out=lo_out[m // 2][:, m % 2, :],
                                in0=raw, in1=hi_ap, op=OP.subtract)

                chunk_raw = {}
                for m in range(MT):
                    wm = wmp.tile([128, n_kp, 2, 128], f8, tag=wm_tag,
                                  name=f"wm_{lname}_{m}")
                    w_read(m, wm)
                    ps = psum.tile([128, B_LOC], f32, tag="mm",
                                   name=f"ps_{lname}_{m}")
                    for kp in range(n_kp):
                        w_ap = wm[:, kp, :, :]
                        for hl, src in srcs:
                            for n in range(2):
                                nc.tensor.matmul(
                                    ps[:, n * 512:(n + 1) * 512],
                                    w_ap,
                                    src[kp][:, :, n * 512:(n + 1) * 512],
                                    start=(kp == 0 and hl == 0),
                                    stop=(kp == n_kp - 1 and hl == last_hl),
                                    perf_mode=PM)
                    raw = hraw[hraw_ctr[0] % HRAW_POOL]
                    hraw_ctr[0] += 1
                    chunk_raw.setdefault(m // CH, []).append(raw)
                    nc.vector.tensor_scalar(
                        out=raw, in0=ps, scalar1=1.0, scalar2=None,
                        op0=OP.mult, op1=OP.add, accum_out=S1[:, m:m + 1])
                    sj = ternp.tile([128, B_LOC], bf16, tag="sq", name="sqj",
                                    bufs=1)
                    nc.scalar.activation(out=sj, in_=raw, func=AF.Square,
                                         accum_out=S2[:, m:m + 1])
                    if m % CH == CH - 1:
                        c = m // CH
                        stats_chunk(c)
                        norm_chunk(c, chunk_raw.pop(c))
                    pump(pump_n)
                pump(len(bg))

            # ================= startup ======================================
            # delta partials for hid0 shard first (gates H0's AllGather)
            wv_h = [whT[l].rearrange("(kl p) f -> kl p f", p=128)
                    for l in range(N_MID)]
            if n_mid_eff > 0:
                h0_slabs = [(wv_h[0][kl][:, mh * 2048:(mh + 1) * 2048], 2048)
                            for kl in range(KL) for mh in range(2)]
                ph0 = delta_reduce(h0_slabs, "h0")
                nc.gpsimd.dma_start(out=dlA_in, in_=ph0)
                nc.gpsimd.collective_compute(
                    "AllReduce", OP.add, replica_groups=RG,
                    ins=[dlA_in.opt()], outs=[dlA_out.opt()])

            # x load + hi/lo cast (feeds input-layer matmuls)
            xv = xT.rearrange("(t p) b -> t p b", p=128)
            for t in range(KT_IN):
                xs = slabp.tile([128, B_LOC], f32, tag="slab", name=f"xs{t}")
                nc.sync.dma_start(out=xs, in_=xv[t])
                hi_ap = XH[t // 2][:, t % 2, :]
                nc.gpsimd.tensor_copy(out=hi_ap, in_=xs)
                nc.vector.tensor_tensor(out=XL[t // 2][:, t % 2, :],
                                        in0=xs, in1=hi_ap, op=OP.subtract)

            # delta for W_in (local, full matrix on every core; skip the
            # all-zero padding k-tile 7)
            wv_in = winT.rearrange("(t p) f -> t p f", p=128)
            in_slabs = [(wv_in[t][:, mh * 2048:(mh + 1) * 2048], 2048)
                        for t in range(KT_IN_NZ) for mh in range(2)]
            pin = delta_reduce(in_slabs, "in")
            inv_in = bcast_delta(pin, KIN * HID, "in")

            # zero-fill the tw_in slots of padding k-tile 7 (kp 3, member 1)
            uz = ternp.tile([128, 2048], f8, tag="u", name="uz")
            nc.vector.memset(uz, 0.0)
            for mh in range(2):
                nc.gpsimd.dma_start(
                    out=tw_in[mh * 16:(mh + 1) * 16, 3, 1].rearrange(
                        "m p c -> p m c"),
                    in_=uz.rearrange("p (m c) -> p m c", c=128))

            # ternarize W_in m-half 0 first so L0 can start, then the hid0
            # shard (gates the first AllGather), then W_in m-half 1.
            def tern_win(mh):
                for t in range(KT_IN_NZ):
                    tern_slab(wv_in[t][:, mh * 2048:(mh + 1) * 2048], inv_in,
                              tw_in[mh * 16:(mh + 1) * 16, t // 2,
                                    t % 2].rearrange("m p c -> p m c"),
                              2048)

            tern_win(0)
            if n_mid_eff > 0:
                ph0g = small.tile([128, 1], f32, tag="ph0g")
                nc.gpsimd.dma_start(out=ph0g, in_=dlA_out)
                inv_h0 = bcast_delta(ph0g, HID * HID, "h0")
                # m-half-major so each AllGather chunk fires as soon as ready
                for mh in range(2):
                    for kl in range(KL):
                        tern_slab(
                            wv_h[0][kl][:, mh * 2048:(mh + 1) * 2048], inv_h0,
                            tw_hid_sh[0, mh * 16:(mh + 1) * 16, kl // 2,
                                      kl % 2].rearrange("m p c -> p m c"),
                            2048)
                    nc.gpsimd.collective_compute(
                        "AllGather", OP.bypass, replica_groups=RG,
                        ins=[tw_hid_sh[0, mh * 16:(mh + 1) * 16].opt()],
                        outs=[tw_hid[0][mh].opt()])
            tern_win(1)

            # after startup, bg work shares queues with the m-loop DMA
            qstate["delta"] = [nc.sync, nc.gpsimd]

            # delta partials for hid1..3 + out -> AllReduce #2.
            # Split into per-slab bg thunks so L0's own DMA stays interleaved.
            invs = {}
            dparts = {}

            def mk_dpart_thunk(l, idx, ap, cols):
                def t():
                    if l not in dparts:
                        dparts[l] = small.tile([128, 16], f32,
                                               tag=f"part_h{l}",
                                               name=f"part_h{l}")
                        nc.vector.memset(dparts[l], 0.0)
                    sl = slabp.tile([128, cols], f32, tag="slab", name="dslab")
                    dq().dma_start(out=sl, in_=ap)
                    nc.vector.tensor_reduce(out=dparts[l][:, idx:idx + 1],
                                            in_=sl, axis=AX.X, op=OP.add,
                                            apply_absolute_value=True)
                return t

            def emit_delta_rest_final():
                pb = small.tile([128, 4], f32, tag="pb")
                nc.vector.memset(pb, 0.0)
                for i, l in enumerate(range(1, n_mid_eff)):
                    nc.vector.tensor_reduce(out=pb[:, i:i + 1], in_=dparts[l],
                                            axis=AX.X, op=OP.add)
                wv_o = woT.rearrange("(s p) c -> s p c", p=128)
                o_slabs = [(wv_o[s], 16) for s in range(4)]
                po = delta_reduce(o_slabs, "out")
                nc.vector.tensor_copy(out=pb[:, 3:4], in_=po)
                nc.gpsimd.dma_start(out=dlB_in, in_=pb)
                nc.gpsimd.collective_compute(
                    "AllReduce", OP.add, replica_groups=RG,
                    ins=[dlB_in.opt()], outs=[dlB_out.opt()])
                pbg = small.tile([128, 4], f32, tag="pbg")
                nc.gpsimd.dma_start(out=pbg, in_=dlB_out)
                for i, l in enumerate(range(1, n_mid_eff)):
                    invs[l] = bcast_delta(pbg[:, i:i + 1], HID * HID, f"h{l}")
                invs["out"] = bcast_delta(pbg[:, 3:4], 10 * HID, "out")

            for l in range(1, n_mid_eff):
                for idx, (kl, mh) in enumerate(
                        (kl, mh) for kl in range(KL) for mh in range(2)):
                    bg.append(mk_dpart_thunk(
                        l, idx, wv_h[l][kl][:, mh * 2048:(mh + 1) * 2048],
                        2048))
            bg.append(emit_delta_rest_final)

            def emit_tern_hid(l):
                # invs[l] is looked up lazily: the delta thunks run first
                for kl in range(KL):
                    for mh in range(2):
                        bg.append(lambda l=l, kl=kl, mh=mh: tern_slab(
                            wv_h[l][kl][:, mh * 2048:(mh + 1) * 2048], invs[l],
                            tw_hid_sh[l, mh * 16:(mh + 1) * 16, kl // 2,
                                      kl % 2].rearrange("m p c -> p m c"),
                            2048))

            def emit_ag_hid(l):
                for mh in range(2):
                    nc.gpsimd.collective_compute(
                        "AllGather", OP.bypass, replica_groups=RG,
                        ins=[tw_hid_sh[l, mh * 16:(mh + 1) * 16].opt()],
                        outs=[tw_hid[l][mh].opt()])

            def emit_tern_out():
                wv_o2 = woT.rearrange("(s p) c -> s p c", p=128)
                for s in range(4):
                    sl = slabp.tile([128, 16], f32, tag="oslab",
                                    name="oslab")
                    nc.gpsimd.dma_start(out=sl, in_=wv_o2[s])
                    u = ternp.tile([128, 16], f8, tag="ou", name="ou")
                    v = ternp.tile([128, 16], f8, tag="ov", name="ov")
                    nc.scalar.activation(out=u, in_=sl, func=AF.Sign,
                                         bias=negone, scale=invs["out"])
                    nc.scalar.activation(out=v, in_=sl, func=AF.Sign,
                                         bias=posone, scale=invs["out"])
                    nc.vector.tensor_tensor(out=u, in0=u, in1=v, op=OP.add)
                    nc.gpsimd.dma_start(out=tw_out_sh[s // 2, s % 2], in_=u)
                nc.gpsimd.collective_compute(
                    "AllGather", OP.bypass, replica_groups=RG,
                    ins=[tw_out_sh.opt()], outs=[tw_out.opt()])

            # ================= layers =======================================
            def w_read_in(m, wm):
                nc.sync.dma_start(
                    out=wm,
                    in_=tw_in[m].rearrange("kp two p c -> p kp two c"))

            def w_read_hid(l):
                def f(m, wm):
                    half = tw_hid[l][m // 16]
                    for r in range(N_CORES):
                        nc.sync.dma_start(
                            out=wm[:, r * 2:(r + 1) * 2, :, :],
                            in_=half[r, m % 16].rearrange(
                                "klp two p c -> p klp two c"))
                return f

            bufs = [(AH, AL), (BH, BL)]

            # tern(1) is pumped during L0; AG(l+1) is emitted right before
            # layer H{l} so it runs during H{l} and is done for H{l+1}.
            if n_mid_eff > 1:
                emit_tern_hid(1)
            if n_mid_eff == 0:
                bg.append(emit_tern_out)
            mm_layer("L0", XH, XL, KP_IN, AH, AL, w_read_in, "wmin",
                     gam_sb[0], bet_sb[0],
                     make_lo=use_lo_in("H0" if n_mid_eff else "out"),
                     pump_n=3)

            for l in range(n_mid_eff):
                if l + 1 < n_mid_eff:
                    emit_ag_hid(l + 1)
                    if l + 2 < n_mid_eff:
                        emit_tern_hid(l + 2)
                    else:
                        bg.append(emit_tern_out)
                hi_in, lo_in = bufs[l % 2]
                hi_out, lo_out = bufs[(l + 1) % 2]
                nxt = f"H{l + 1}" if l + 1 < n_mid_eff else "out"
                mm_layer(f"H{l}", hi_in, lo_in, KP_H, hi_out, lo_out,
                         w_read_hid(l), "wm", gam_sb[l + 1], bet_sb[l + 1],
                         make_lo=use_lo_in(nxt))

            # ================= output layer + log-softmax ===================
            hi_fin, lo_fin = bufs[n_mid_eff % 2]
            wmo = wmp.tile([128, KP_H, 2, 16], f8, tag="wmo", name="wmo")
            nc.sync.dma_start(
                out=wmo,
                in_=tw_out.rearrange("r klp two p c -> p (r klp) two c"))
            pso = psum.tile([16, B_LOC], f32, tag="mm", name="pso")
            osrcs = [(0, hi_fin)]
            if use_lo_in("out"):
                osrcs.append((1, lo_fin))
            olast = osrcs[-1][0]
            for kp in range(KP_H):
                w_ap = wmo[:, kp, :, :]
                for hl, src in osrcs:
                    for n in range(2):
                        nc.tensor.matmul(
                            pso[:, n * 512:(n + 1) * 512],
                            w_ap,
                            src[kp][:, :, n * 512:(n + 1) * 512],
                            start=(kp == 0 and hl == 0),
                            stop=(kp == KP_H - 1 and hl == olast),
                            perf_mode=PM)
            s12o = stats.tile([10, 2], f32, tag="s12o")
            opre = small.tile([10, B_LOC], f32, tag="opre")
            nc.vector.tensor_scalar(out=opre, in0=pso[0:10, :], scalar1=1.0,
                                    scalar2=None, op0=OP.mult, op1=OP.add,
                                    accum_out=s12o[:, 0:1])
            sjo = ternp.tile([10, B_LOC], bf16, tag="sq", name="sqo", bufs=1)
            nc.scalar.activation(out=sjo, in_=pso[0:10, :], func=AF.Square,
                                 accum_out=s12o[:, 1:2])
            bno_in = dram.tile([10, 2], f32)
            bno_out = dram.tile([10, 2], f32, addr_space="Shared")
            nc.gpsimd.dma_start(out=bno_in, in_=s12o)
            nc.gpsimd.collective_compute(
                "AllReduce", OP.add, replica_groups=RG,
                ins=[bno_in.opt()], outs=[bno_out.opt()])
            sgo = stats.tile([10, 2], f32, tag="sgo")
            nc.gpsimd.dma_start(out=sgo, in_=bno_out)
            meano = stats.tile([10, 1], f32, tag="meano")
            nc.vector.tensor_scalar_mul(meano, sgo[:, 0:1], 1.0 / B)
            ex2o = stats.tile([10, 1], f32, tag="ex2o")
            nc.vector.tensor_scalar_mul(ex2o, sgo[:, 1:2], 1.0 / B)
            msqo = stats.tile([10, 1], f32, tag="msqo")
            nc.vector.tensor_tensor(out=msqo, in0=meano, in1=meano, op=OP.mult)
            varo = stats.tile([10, 1], f32, tag="varo")
            nc.vector.tensor_tensor(out=varo, in0=ex2o, in1=msqo,
                                    op=OP.subtract)
            sdo = stats.tile([10, 1], f32, tag="sdo")
            nc.scalar.activation(out=sdo, in_=varo, func=AF.Sqrt,
                                 bias=epsb[0:10, :])
            rso = stats.tile([10, 1], f32, tag="rso")
            nc.vector.reciprocal(out=rso, in_=sdo)
            sclo = stats.tile([10, 1], f32, tag="sclo")
            nc.vector.tensor_tensor(out=sclo, in0=rso, in1=go_sb, op=OP.mult)
            mso = stats.tile([10, 1], f32, tag="mso")
            nc.vector.tensor_tensor(out=mso, in0=meano, in1=sclo, op=OP.mult)
            biao = stats.tile([10, 1], f32, tag="biao")
            nc.vector.tensor_tensor(out=biao, in0=bo_sb, in1=mso,
                                    op=OP.subtract)
            onorm = opre
            nc.scalar.activation(out=onorm, in_=opre, func=AF.Identity,
                                 bias=biao, scale=sclo)
            esb = small.tile([10, B_LOC], f32, tag="esb")
            nc.scalar.activation(out=esb, in_=onorm, func=AF.Exp)
            csp = psum.tile([1, B_LOC], f32, tag="cs", bufs=1)
            for n in range(2):
                nc.tensor.matmul(csp[:, n * 512:(n + 1) * 512], ones10,
                                 esb[:, n * 512:(n + 1) * 512],
                                 start=True, stop=True)
            lsb = small.tile([1, B_LOC], f32, tag="lsb")
            nc.scalar.activation(out=lsb, in_=csp, func=AF.Ln)
            lse_bc = psum.tile([10, B_LOC], f32, tag="cs", name="lse_bc",
                               bufs=1)
            for n in range(2):
                nc.tensor.matmul(lse_bc[:, n * 512:(n + 1) * 512], ones1x10,
                                 lsb[:, n * 512:(n + 1) * 512],
                                 start=True, stop=True)
            fout = small.tile([10, B_LOC], f32, tag="esb", name="fout")
            nc.vector.tensor_tensor(out=fout, in0=onorm, in1=lse_bc,
                                    op=OP.subtract)
            nc.gpsimd.dma_start(out=out[:], in_=fout)

    nc.compile()
    return nc


def _get_program():
    if "nc" not in _cache:
        _cache["nc"] = _build()
    return _cache["nc"]


def kernel(x, W_in, b_in, W_hid, b_hid, W_out, b_out, gamma, beta,
           gamma_out, beta_out):
    from concourse.bass_utils import run_bass_kernel_spmd

    nc = _get_program()

    x = np.asarray(x, dtype=np.float32).reshape(B, KIN)
    # layout-only host prep (transpose + zero-pad + shard)
    xT_full = np.zeros((KIN_PAD, B), dtype=np.float32)
    xT_full[:KIN] = x.T
    winT_full = np.zeros((KIN_PAD, HID), dtype=np.float32)
    winT_full[:KIN] = np.asarray(W_in, dtype=np.float32).T
    whT_full = np.ascontiguousarray(
        np.asarray(W_hid, dtype=np.float32).transpose(0, 2, 1))
    woT_full = np.zeros((HID, 16), dtype=np.float32)
    woT_full[:, :10] = np.asarray(W_out, dtype=np.float32).T
    gam_np = np.ascontiguousarray(np.asarray(gamma, dtype=np.float32))
    bet_np = np.ascontiguousarray(np.asarray(beta, dtype=np.float32))
    gob_np = np.stack([np.asarray(gamma_out, dtype=np.float32),
                       np.asarray(beta_out, dtype=np.float32)])

    in_maps = []
    for c in range(N_CORES):
        in_maps.append({
            "xT": np.ascontiguousarray(
                xT_full[:, c * B_LOC:(c + 1) * B_LOC]),
            "winT": winT_full,
            "whT": np.ascontiguousarray(
                whT_full[:, c * SH_H:(c + 1) * SH_H, :]),
            "woT": np.ascontiguousarray(
                woT_full[c * SH_H:(c + 1) * SH_H, :]),
            "gam": gam_np,
            "bet": bet_np,
            "gob": gob_np,
        })

    res = run_bass_kernel_spmd(nc, in_maps, core_ids=list(range(N_CORES)))
    return np.concatenate(
        [np.ascontiguousarray(res.results[c]["out"].T) for c in range(N_CORES)],
        axis=0)


# revision 28
# speedup vs baseline: 1.3226x; 1.0679x over previous
"""Trainium2 Bass kernel for nn_MnistNet (ternary-weight MLP with training-mode
BatchNorm), data-parallel over batch across 8 NeuronCores.

Strategy (v3: fp8 DoubleRow, multi-queue DMA, partial hi/lo)
------------------------------------------------------------
- Host side does layout-only prep: transpose x / weights, zero-pad 784->1024,
  shard the batch (1024 rows/core) and the weight rows (1/8 per core).
- All matmuls run in fp8e4 with MatmulPerfMode.DoubleRow (2 k-tiles per
  instruction; measured on hw: same 262ns cadence as a bf16 matmul, so 2x
  throughput per instruction).  Ternary weights {-2,0,+2} are exact in fp8e4.
- Activations use a hi/lo split (a ~ q8(a) + q8(a - q8(a))) accumulated into
  the same PSUM group where accuracy requires it; layers H2/H3 run plain fp8
  (hi only), which halves their matmul count.  Simulated end-to-end norm-rel
  8.9e-3 (hw-calibrated ~1.0e-2) vs the 2e-2 gate.
- ternarize: t = Sign(w/delta - 1) + Sign(w/delta + 1) in {-2, 0, +2} fp8.
  The 2x scale is washed out by the following BatchNorm.  delta = 0.7*mean|W|
  via per-core partial |.| row-sums + one tiny AllReduce + a ones-matmul
  broadcast.  Biases are dropped entirely (BN subtracts the mean).
- Startup DMA (delta + ternarize passes over the f32 weights) is issued as
  [128, 2048] slabs round-robined over the sync/tensor/gpsimd queues: a
  single queue sustains only ~112 GB/s, so concurrency across queues is
  what approaches the HBM roofline.  The all-zero padding k-tile of W_in is
  never read (memset + store instead).
- Each core ternarizes 1/8 of each hidden/output weight matrix into a
  DoubleRow-tiled fp8 layout and AllGathers it in 2 chunks.  The pipeline is
  2 layers deep: tern(l+1) runs during layer l-1's matmuls (via the bg pump),
  AllGather(l+1) runs during layer l, so layer l+1 never waits and the CC
  queue stays clear for the BN-stats AllReduces.
- Activations kept transposed (features on partitions, batch on free dim).
  BN stats are free-dim reductions fused into the PSUM drains, AllReduced in
  8 chunks of 4 m-tiles per layer so only the last chunk's latency is
  exposed.  Pre-BN activations live in a rotating 12-tile bf16 pool; the
  sum-of-squares pass reads those bf16 tiles (2x DVE rate vs f32 PSUM).
- BN apply: ACT scale/bias pass, DVE hardtanh clamp, Pool-engine cast to fp8
  (hi), DVE subtract for the residual (lo, only where consumed).
- log-softmax: exp on ACT, partition-sum and broadcast via ones-matmuls on
  the PE (no DRAM round-trip), subtract, DMA out.
"""

import os

import numpy as np

N_CORES = 8
B = 8192
B_LOC = B // N_CORES          # 1024 rows per core
HID = 4096
N_MID = 4
KIN = 784
KIN_PAD = 1024                # 8 * 128 (even k-tiles for DoubleRow pairs)
KT_IN = KIN_PAD // 128        # 8
KT_IN_NZ = 7                  # k-tiles with any nonzero data (784 < 7*128)
KP_IN = KT_IN // 2            # 4 k-pairs
KT_H = HID // 128             # 32
KP_H = KT_H // 2              # 16 k-pairs
MT = HID // 128               # 32 output-feature tiles
KL = 4                        # k-tiles per core shard of a hidden layer
SH_H = 512                    # hidden-weight rows per core (4096/8)
EPS = 1e-5
RG = [list(range(N_CORES))]
CH = 4                        # m-tiles per BN-stats chunk
NCH = MT // CH                # 8 chunks per layer
HRAW_POOL = 12                # rotating pre-BN bf16 tiles (3 chunks)
# fp8 hi/lo schedule: layers H1/H2/H3 run plain fp8 (their inputs skip the
# lo residual matmuls); L0/H0/out keep the hi/lo split.  Simulated end-to-end
# norm-rel 1.17e-2 vs the 2e-2 gate (hw tracked sim within 1%).
USE_LO_IN = {"L0": True, "H0": True, "H1": False, "H2": False, "H3": False,
             "out": True}

_cache = {}


def _build():
    n_mid_eff = int(os.environ.get("KERNEL_NMID", str(N_MID)))
    import concourse.bass as bass
    import concourse.bacc as bacc
    import concourse.mybir as mybir
    import concourse.tile as tile

    f32 = mybir.dt.float32
    bf16 = mybir.dt.bfloat16
    f8 = mybir.dt.float8e4
    AX = mybir.AxisListType
    OP = mybir.AluOpType
    AF = mybir.ActivationFunctionType
    PM = mybir.MatmulPerfMode.DoubleRow

    nc = bacc.Bacc("TRN2", target_bir_lowering=False, debug=False,
                   num_devices=N_CORES)

    def use_lo_in(lname):
        return USE_LO_IN.get(lname, True) and n_mid_eff == N_MID

    # ---- I/O ----------------------------------------------------------------
    xT = nc.dram_tensor("xT", [KIN_PAD, B_LOC], f32, kind="ExternalInput")
    winT = nc.dram_tensor("winT", [KIN_PAD, HID], f32, kind="ExternalInput")
    whT = nc.dram_tensor("whT", [N_MID, SH_H, HID], f32, kind="ExternalInput")
    woT = nc.dram_tensor("woT", [SH_H, 16], f32, kind="ExternalInput")
    gam = nc.dram_tensor("gam", [N_MID + 1, HID], f32, kind="ExternalInput")
    bet = nc.dram_tensor("bet", [N_MID + 1, HID], f32, kind="ExternalInput")
    gob = nc.dram_tensor("gob", [2, 10], f32, kind="ExternalInput")
    # [10, B_LOC] layout: transposed on host (interleaved-partition DRAM
    # stores kill the device).
    out = nc.dram_tensor("out", [10, B_LOC], f32, kind="ExternalOutput")

    with tile.TileContext(nc) as tc:
        with (
            tc.tile_pool(name="ht", bufs=1) as ht,
            tc.tile_pool(name="hrawp", bufs=1) as hrawp,
            tc.tile_pool(name="wmp", bufs=2) as wmp,
            tc.tile_pool(name="slab", bufs=2) as slabp,
            tc.tile_pool(name="tern", bufs=2) as ternp,
            tc.tile_pool(name="small", bufs=1) as small,
            tc.tile_pool(name="stats", bufs=2) as stats,
            tc.tile_pool(name="psum", bufs=2, space="PSUM") as psum,
            tc.tile_pool(name="dram", bufs=1, space="DRAM") as dram,
        ):
            # DMA queue rotation: only sync/scalar/gpsimd can initiate DMAs.
            # Delta-pass loads may use the scalar queue (idle then); the
            # ternarize loads avoid it (it runs the Sign ops).
            qstate = {"delta": [nc.sync, nc.scalar, nc.gpsimd],
                      "tern": [nc.sync, nc.scalar], "i": 0}

            def dq(kind="delta"):
                qstate["i"] += 1
                qs = qstate[kind]
                return qs[qstate["i"] % len(qs)]

            # ---- persistent small tiles ------------------------------------
            ones128 = small.tile([128, 128], f32, tag="ones128")
            nc.vector.memset(ones128, 1.0)
            ones10 = small.tile([10, 1], f32, tag="ones10")
            nc.vector.memset(ones10, 1.0)
            ones1x10 = small.tile([1, 10], f32, tag="ones1x10")
            nc.vector.memset(ones1x10, 1.0)
            negone = small.tile([128, 1], f32, tag="negone")
            nc.vector.memset(negone, -1.0)
            posone = small.tile([128, 1], f32, tag="posone")
            nc.vector.memset(posone, 1.0)
            epsb = small.tile([128, 1], f32, tag="epsb")
            nc.vector.memset(epsb, EPS)

            gam_sb, bet_sb = [], []
            for l in range(N_MID + 1):
                g = small.tile([128, MT], f32, tag=f"gam{l}", name=f"gam_sb{l}")
                nc.gpsimd.dma_start(out=g, in_=gam[l].rearrange("(m p) -> p m", p=128))
                gam_sb.append(g)
                b = small.tile([128, MT], f32, tag=f"bet{l}", name=f"bet_sb{l}")
                nc.gpsimd.dma_start(out=b, in_=bet[l].rearrange("(m p) -> p m", p=128))
                bet_sb.append(b)
            go_sb = small.tile([10, 1], f32, tag="go")
            nc.gpsimd.dma_start(out=go_sb, in_=gob[0:1, :].rearrange("a f -> f a"))
            bo_sb = small.tile([10, 1], f32, tag="bo")
            nc.gpsimd.dma_start(out=bo_sb, in_=gob[1:2, :].rearrange("a f -> f a"))

            # ---- activation double buffers (fp8 hi/lo k-pair tiles) --------
            # L0/H1/H3 write the A set; H0/H2 write the B set.  x aliases the
            # B set (its first writer is H0's norm pass, which runs after L0
            # has fully consumed x).
            AH = [ht.tile([128, 2, B_LOC], f8, tag=f"AH{p}", name=f"AH{p}")
                  for p in range(KP_H)]
            AL = [ht.tile([128, 2, B_LOC], f8, tag=f"AL{p}", name=f"AL{p}")
                  for p in range(KP_H)]
            BH = [ht.tile([128, 2, B_LOC], f8, tag=f"BH{p}", name=f"BH{p}")
                  for p in range(KP_H)]
            BL = [ht.tile([128, 2, B_LOC], f8, tag=f"BL{p}", name=f"BL{p}")
                  for p in range(KP_H)]
            XH = BH[:KP_IN]
            XL = BL[:KP_IN]

            # pre-BN bf16 scratch, rotating pool
            hraw = [hrawp.tile([128, B_LOC], bf16, tag=f"hr{j}", name=f"hr{j}")
                    for j in range(HRAW_POOL)]
            hraw_ctr = [0]

            # ---- DRAM scratch ----------------------------------------------
            tw_in = dram.tile([MT, KP_IN, 2, 128, 128], f8)
            tw_hid_sh = dram.tile([N_MID, MT, 2, 2, 128, 128], f8)
            # gathered quarters: [rank, m-chunk 8, klp 2, two 2, 128, 128].
            # Quartered so each AllGather slots between BN-stats AllReduces
            # on the in-order CC queue without head-of-line blocking them.
            tw_hid = [[dram.tile([N_CORES, MT // 4, 2, 2, 128, 128], f8,
                                 addr_space="Shared", tag=f"twq{l}_{q}",
                                 name=f"twq{l}_{q}")
                       for q in range(4)] for l in range(N_MID)]
            tw_out_sh = dram.tile([2, 2, 128, 16], f8)
            tw_out = dram.tile([N_CORES, 2, 2, 128, 16], f8,
                               addr_space="Shared")
            dlA_in = dram.tile([128, 1], f32)
            dlA_out = dram.tile([128, 1], f32, addr_space="Shared")
            dlB_in = dram.tile([128, 4], f32)
            dlB_out = dram.tile([128, 4], f32, addr_space="Shared")

            # ---- helpers ----------------------------------------------------
            def bcast_delta(partial_col, n_elems, nm):
                """[128,1] per-partition partial |W| sums -> broadcasted
                1/delta [128,1] (all partitions equal)."""
                ps = psum.tile([128, 1], f32, tag="small", name=f"dps_{nm}",
                               bufs=1)
                nc.tensor.matmul(ps, ones128, partial_col, start=True, stop=True)
                dsc = small.tile([128, 1], f32, tag=f"dsc_{nm}")
                nc.scalar.activation(out=dsc, in_=ps, func=AF.Copy,
                                     scale=0.7 / float(n_elems))
                inv = small.tile([128, 1], f32, tag=f"inv_{nm}")
                nc.vector.reciprocal(out=inv, in_=dsc)
                return inv

            def tern_slab(src_ap, inv_ap, dst_ap, cols):
                """ternarize one [128, cols] f32 slab -> {-2,0,2} fp8 in DRAM.
                dst_ap must be a [128, cols//128, 128] view."""
                sl = slabp.tile([128, cols], f32, tag="slab", name="tslab")
                dq("tern").dma_start(out=sl, in_=src_ap)
                u = ternp.tile([128, cols], f8, tag="u", name="ternu")
                v = ternp.tile([128, cols], f8, tag="v", name="ternv")
                nc.scalar.activation(out=u, in_=sl, func=AF.Sign,
                                     bias=negone, scale=inv_ap)
                nc.scalar.activation(out=v, in_=sl, func=AF.Sign,
                                     bias=posone, scale=inv_ap)
                nc.vector.tensor_tensor(out=u, in0=u, in1=v, op=OP.add)
                nc.gpsimd.dma_start(out=dst_ap,
                                    in_=u.rearrange("p (m c) -> p m c", c=128))

            def delta_reduce(src_slabs, nm):
                """abs row-sum partials of a list of slab APs -> [128,1]."""
                part = small.tile([128, 16], f32, tag=f"part_{nm}",
                                  name=f"part_{nm}")
                nc.vector.memset(part, 0.0)
                for s, (ap, cols) in enumerate(src_slabs):
                    sl = slabp.tile([128, cols], f32, tag="slab", name="dslab")
                    dq().dma_start(out=sl, in_=ap)
                    nc.vector.tensor_reduce(out=part[:, s:s + 1], in_=sl,
                                            axis=AX.X, op=OP.add,
                                            apply_absolute_value=True)
                tot = small.tile([128, 1], f32, tag=f"ptot_{nm}",
                                 name=f"ptot_{nm}")
                nc.vector.tensor_reduce(out=tot, in_=part, axis=AX.X, op=OP.add)
                return tot

            # background work queue: thunks emitted interleaved into m-loops
            bg = []

            def pump(n=1):
                for _ in range(min(n, len(bg))):
                    bg.pop(0)()

            # ---- layer runner ----------------------------------------------
            def mm_layer(lname, hi_in, lo_in, n_kp, hi_out, lo_out,
                         w_read, wm_tag, gam_l, bet_l, make_lo, pump_n=2,
                         cc=None):
                """One ternary-linear + BN + hardtanh layer, fp8 DoubleRow.

                hi_in/lo_in: [128, 2, B_LOC] fp8 k-pair tiles (input).
                hi_out/lo_out: same for the output (lo skipped if not make_lo).
                w_read(m, wm): fill wm [128, n_kp, 2, 128] fp8 for m-tile m.
                """
                srcs = [(0, hi_in)]
                if use_lo_in(lname):
                    srcs.append((1, lo_in))
                last_hl = srcs[-1][0]
                S1 = stats.tile([128, MT], f32, tag="s1", name=f"S1_{lname}")
                S2 = stats.tile([128, MT], f32, tag="s2", name=f"S2_{lname}")
                sg = [None] * NCH

                def stats_chunk(c):
                    # AllReduce (sum, sumsq) for m-tiles [c*CH, (c+1)*CH)
                    bin_ = dram.tile([128, 2 * CH], f32,
                                     tag=f"bns_in_{lname}_{c}",
                                     name=f"bns_in_{lname}_{c}")
                    bout_ = dram.tile([128, 2 * CH], f32, addr_space="Shared",
                                      tag=f"bns_out_{lname}_{c}",
                                      name=f"bns_out_{lname}_{c}")
                    nc.gpsimd.dma_start(out=bin_[:, 0:CH],
                                        in_=S1[:, c * CH:(c + 1) * CH])
                    nc.gpsimd.dma_start(out=bin_[:, CH:2 * CH],
                                        in_=S2[:, c * CH:(c + 1) * CH])
                    nc.gpsimd.collective_compute(
                        "AllReduce", OP.add, replica_groups=RG,
                        ins=[bin_.opt()], outs=[bout_.opt()])
                    g = stats.tile([128, 2 * CH], f32, tag=f"sg{c}",
                                   name=f"sg_{lname}_{c}")
                    nc.gpsimd.dma_start(out=g, in_=bout_)
                    sg[c] = g

                def norm_chunk(c, raw_tiles):
                    # scale = gamma*rsqrt(var+eps); bias = beta - mean*scale
                    g = sg[c]
                    mean = stats.tile([128, CH], f32, tag="mean",
                                      name=f"mean_{lname}_{c}")
                    nc.vector.tensor_scalar_mul(mean, g[:, 0:CH], 1.0 / B)
                    ex2 = stats.tile([128, CH], f32, tag="ex2",
                                     name=f"ex2_{lname}_{c}")
                    nc.vector.tensor_scalar_mul(ex2, g[:, CH:2 * CH], 1.0 / B)
                    var = stats.tile([128, CH], f32, tag="var",
                                     name=f"var_{lname}_{c}")
                    nc.vector.tensor_tensor(out=var, in0=mean, in1=mean,
                                            op=OP.mult)
                    nc.vector.tensor_tensor(out=var, in0=ex2, in1=var,
                                            op=OP.subtract)
                    sd = stats.tile([128, CH], f32, tag="sd",
                                    name=f"sd_{lname}_{c}")
                    nc.scalar.activation(out=sd, in_=var, func=AF.Sqrt,
                                         bias=epsb)
                    rs = stats.tile([128, CH], f32, tag="rs",
                                    name=f"rs_{lname}_{c}")
                    nc.vector.reciprocal(out=rs, in_=sd)
                    scl = stats.tile([128, CH], f32, tag="scl",
                                     name=f"scl_{lname}_{c}")
                    nc.vector.tensor_tensor(out=scl, in0=rs,
                                            in1=gam_l[:, c * CH:(c + 1) * CH],
                                            op=OP.mult)
                    bia = stats.tile([128, CH], f32, tag="bia",
                                     name=f"bia_{lname}_{c}")
                    nc.vector.tensor_tensor(out=bia, in0=mean, in1=scl,
                                            op=OP.mult)
                    nc.vector.tensor_tensor(out=bia,
                                            in0=bet_l[:, c * CH:(c + 1) * CH],
                                            in1=bia, op=OP.subtract)
                    for j in range(CH):
                        m = c * CH + j
                        raw = raw_tiles[j]
                        # normalize (ACT), hardtanh clamp in place (DVE)
                        nc.scalar.activation(out=raw, in_=raw,
                                             func=AF.Identity,
                                             bias=bia[:, j:j + 1],
                                             scale=scl[:, j:j + 1])
                        nc.vector.tensor_scalar(
                            out=raw, in0=raw, scalar1=1.0, scalar2=-1.0,
                            op0=OP.min, op1=OP.max)
                        # hi = fp8(raw) on Pool; lo = raw - hi on DVE
                        hi_ap = hi_out[m // 2][:, m % 2, :]
                        nc.gpsimd.tensor_copy(out=hi_ap, in_=raw)
                        if make_lo:
                            nc.vector.tensor_tensor(
                                out=lo_out[m // 2][:, m % 2, :],
                                in0=raw, in1=hi_ap, op=OP.subtract)

                chunk_raw = {}
                for m in range(MT):
                    wm = wmp.tile([128, n_kp, 2, 128], f8, tag=wm_tag,
                                  name=f"wm_{lname}_{m}")
                    w_read(m, wm)
                    ps = psum.tile([128, B_LOC], f32, tag="mm",
                                   name=f"ps_{lname}_{m}")
                    for kp in range(n_kp):
                        w_ap = wm[:, kp, :, :]
                        for hl, src in srcs:
                            for n in range(2):
                                nc.tensor.matmul(
                                    ps[:, n * 512:(n + 1) * 512],
                                    w_ap,
                                    src[kp][:, :, n * 512:(n + 1) * 512],
                                    start=(kp == 0 and hl == 0),
                                    stop=(kp == n_kp - 1 and hl == last_hl),
                                    perf_mode=PM)
                    raw = hraw[hraw_ctr[0] % HRAW_POOL]
                    hraw_ctr[0] += 1
                    chunk_raw.setdefault(m // CH, []).append(raw)
                    nc.vector.tensor_scalar(
                        out=raw, in0=ps, scalar1=1.0, scalar2=None,
                        op0=OP.mult, op1=OP.add, accum_out=S1[:, m:m + 1])
                    sj = ternp.tile([128, B_LOC], bf16, tag="sq", name="sqj",
                                    bufs=1)
                    nc.scalar.activation(out=sj, in_=raw, func=AF.Square,
                                         accum_out=S2[:, m:m + 1])
                    if m % CH == CH - 1:
                        c = m // CH
                        stats_chunk(c)
                        if cc and cc.get(c):
                            cc[c]()
                        norm_chunk(c, chunk_raw.pop(c))
                    pump(pump_n)
                pump(len(bg))

            # ================= startup ======================================
            # delta partials for hid0 shard first (gates H0's AllGather)
            wv_h = [whT[l].rearrange("(kl p) f -> kl p f", p=128)
                    for l in range(N_MID)]
            if n_mid_eff > 0:
                h0_slabs = [(wv_h[0][kl][:, mh * 2048:(mh + 1) * 2048], 2048)
                            for kl in range(KL) for mh in range(2)]
                ph0 = delta_reduce(h0_slabs, "h0")
                nc.gpsimd.dma_start(out=dlA_in, in_=ph0)
                nc.gpsimd.collective_compute(
                    "AllReduce", OP.add, replica_groups=RG,
                    ins=[dlA_in.opt()], outs=[dlA_out.opt()])

            # delta for W_in (local, full matrix on every core; skip the
            # all-zero padding k-tile 7)
            wv_in = winT.rearrange("(t p) f -> t p f", p=128)
            in_slabs = [(wv_in[t][:, mh * 2048:(mh + 1) * 2048], 2048)
                        for t in range(KT_IN_NZ) for mh in range(2)]
            pin = delta_reduce(in_slabs, "in")
            inv_in = bcast_delta(pin, KIN * HID, "in")

            # x load + hi/lo cast (feeds input-layer matmuls; deliberately
            # after the delta pass so it doesn't crowd the critical DMA)
            xv = xT.rearrange("(t p) b -> t p b", p=128)
            for t in range(KT_IN):
                xs = slabp.tile([128, B_LOC], f32, tag="slab", name=f"xs{t}")
                nc.sync.dma_start(out=xs, in_=xv[t])
                hi_ap = XH[t // 2][:, t % 2, :]
                nc.gpsimd.tensor_copy(out=hi_ap, in_=xs)
                nc.vector.tensor_tensor(out=XL[t // 2][:, t % 2, :],
                                        in0=xs, in1=hi_ap, op=OP.subtract)

            # zero-fill the tw_in slots of padding k-tile 7 (kp 3, member 1)
            uz = ternp.tile([128, 2048], f8, tag="u", name="uz")
            nc.vector.memset(uz, 0.0)
            for mh in range(2):
                nc.gpsimd.dma_start(
                    out=tw_in[mh * 16:(mh + 1) * 16, 3, 1].rearrange(
                        "m p c -> p m c"),
                    in_=uz.rearrange("p (m c) -> p m c", c=128))

            # ternarize W_in m-half 0 first so L0 can start, then the hid0
            # shard (gates the first AllGather), then W_in m-half 1.
            def tern_win(mh):
                for t in range(KT_IN_NZ):
                    tern_slab(wv_in[t][:, mh * 2048:(mh + 1) * 2048], inv_in,
                              tw_in[mh * 16:(mh + 1) * 16, t // 2,
                                    t % 2].rearrange("m p c -> p m c"),
                              2048)

            tern_win(0)
            if n_mid_eff > 0:
                ph0g = small.tile([128, 1], f32, tag="ph0g")
                nc.gpsimd.dma_start(out=ph0g, in_=dlA_out)
                inv_h0 = bcast_delta(ph0g, HID * HID, "h0")
                # (the AllGather itself is interleaved into L0's CC slots)
                for mh in range(2):
                    for kl in range(KL):
                        tern_slab(
                            wv_h[0][kl][:, mh * 2048:(mh + 1) * 2048], inv_h0,
                            tw_hid_sh[0, mh * 16:(mh + 1) * 16, kl // 2,
                                      kl % 2].rearrange("m p c -> p m c"),
                            2048)
            tern_win(1)

            # after startup, bg work shares queues with the m-loop DMA
            # (the scalar queue is busy with Square/norm during layers)
            qstate["delta"] = [nc.sync, nc.gpsimd]
            qstate["tern"] = [nc.sync, nc.gpsimd]

            # delta partials for hid1..3 + out -> AllReduce #2.
            # Split into per-slab bg thunks so L0's own DMA stays interleaved.
            invs = {}
            dparts = {}

            def mk_dpart_thunk(l, idx, ap, cols):
                def t():
                    if l not in dparts:
                        dparts[l] = small.tile([128, 16], f32,
                                               tag=f"part_h{l}",
                                               name=f"part_h{l}")
                        nc.vector.memset(dparts[l], 0.0)
                    sl = slabp.tile([128, cols], f32, tag="slab", name="dslab")
                    dq().dma_start(out=sl, in_=ap)
                    nc.vector.tensor_reduce(out=dparts[l][:, idx:idx + 1],
                                            in_=sl, axis=AX.X, op=OP.add,
                                            apply_absolute_value=True)
                return t

            def emit_delta_rest_final():
                pb = small.tile([128, 4], f32, tag="pb")
                nc.vector.memset(pb, 0.0)
                for i, l in enumerate(range(1, n_mid_eff)):
                    nc.vector.tensor_reduce(out=pb[:, i:i + 1], in_=dparts[l],
                                            axis=AX.X, op=OP.add)
                wv_o = woT.rearrange("(s p) c -> s p c", p=128)
                o_slabs = [(wv_o[s], 16) for s in range(4)]
                po = delta_reduce(o_slabs, "out")
                nc.vector.tensor_copy(out=pb[:, 3:4], in_=po)
                nc.gpsimd.dma_start(out=dlB_in, in_=pb)
                nc.gpsimd.collective_compute(
                    "AllReduce", OP.add, replica_groups=RG,
                    ins=[dlB_in.opt()], outs=[dlB_out.opt()])
                pbg = small.tile([128, 4], f32, tag="pbg")
                nc.gpsimd.dma_start(out=pbg, in_=dlB_out)
                for i, l in enumerate(range(1, n_mid_eff)):
                    invs[l] = bcast_delta(pbg[:, i:i + 1], HID * HID, f"h{l}")
                invs["out"] = bcast_delta(pbg[:, 3:4], 10 * HID, "out")

            for l in range(1, n_mid_eff):
                for idx, (kl, mh) in enumerate(
                        (kl, mh) for kl in range(KL) for mh in range(2)):
                    bg.append(mk_dpart_thunk(
                        l, idx, wv_h[l][kl][:, mh * 2048:(mh + 1) * 2048],
                        2048))
            bg.append(emit_delta_rest_final)

            def emit_tern_hid(l):
                # invs[l] is looked up lazily: the delta thunks run first
                for kl in range(KL):
                    for mh in range(2):
                        bg.append(lambda l=l, kl=kl, mh=mh: tern_slab(
                            wv_h[l][kl][:, mh * 2048:(mh + 1) * 2048], invs[l],
                            tw_hid_sh[l, mh * 16:(mh + 1) * 16, kl // 2,
                                      kl % 2].rearrange("m p c -> p m c"),
                            2048))

            def ag_hid_thunks(l):
                def mk(q):
                    def t():
                        nc.gpsimd.collective_compute(
                            "AllGather", OP.bypass, replica_groups=RG,
                            ins=[tw_hid_sh[l, q * 8:(q + 1) * 8].opt()],
                            outs=[tw_hid[l][q].opt()])
                    return t
                return [mk(q) for q in range(4)]

            def emit_tern_out():
                wv_o2 = woT.rearrange("(s p) c -> s p c", p=128)
                for s in range(4):
                    sl = slabp.tile([128, 16], f32, tag="oslab",
                                    name="oslab")
                    nc.gpsimd.dma_start(out=sl, in_=wv_o2[s])
                    u = ternp.tile([128, 16], f8, tag="ou", name="ou")
                    v = ternp.tile([128, 16], f8, tag="ov", name="ov")
                    nc.scalar.activation(out=u, in_=sl, func=AF.Sign,
                                         bias=negone, scale=invs["out"])
                    nc.scalar.activation(out=v, in_=sl, func=AF.Sign,
                                         bias=posone, scale=invs["out"])
                    nc.vector.tensor_tensor(out=u, in0=u, in1=v, op=OP.add)
                    nc.gpsimd.dma_start(out=tw_out_sh[s // 2, s % 2], in_=u)
                nc.gpsimd.collective_compute(
                    "AllGather", OP.bypass, replica_groups=RG,
                    ins=[tw_out_sh.opt()], outs=[tw_out.opt()])

            # ================= layers =======================================
            def w_read_in(m, wm):
                nc.sync.dma_start(
                    out=wm,
                    in_=tw_in[m].rearrange("kp two p c -> p kp two c"))

            def w_read_hid(l):
                def f(m, wm):
                    quart = tw_hid[l][m // 8]
                    for r in range(N_CORES):
                        nc.sync.dma_start(
                            out=wm[:, r * 2:(r + 1) * 2, :, :],
                            in_=quart[r, m % 8].rearrange(
                                "klp two p c -> p klp two c"))
                return f

            bufs = [(AH, AL), (BH, BL)]

            # tern(1) is pumped during L0; AG(l+1) is emitted right before
            # layer H{l} so it runs during H{l} and is done for H{l+1}.
            if n_mid_eff > 1:
                emit_tern_hid(1)
            if n_mid_eff == 0:
                bg.append(emit_tern_out)
            # AG(0) quarters ride L0's late CC slots (tern(0) finishes
            # mid-L0); AG(l+1) quarters ride H{l}'s early CC slots.
            cc0 = {4 + q: t for q, t in enumerate(ag_hid_thunks(0))} \
                if n_mid_eff > 0 else None
            mm_layer("L0", XH, XL, KP_IN, AH, AL, w_read_in, "wmin",
                     gam_sb[0], bet_sb[0],
                     make_lo=use_lo_in("H0" if n_mid_eff else "out"),
                     pump_n=3, cc=cc0)

            for l in range(n_mid_eff):
                ccl = None
                if l + 1 < n_mid_eff:
                    ccl = dict(enumerate(ag_hid_thunks(l + 1)))
                    if l + 2 < n_mid_eff:
                        emit_tern_hid(l + 2)
                    else:
                        bg.append(emit_tern_out)
                hi_in, lo_in = bufs[l % 2]
                hi_out, lo_out = bufs[(l + 1) % 2]
                nxt = f"H{l + 1}" if l + 1 < n_mid_eff else "out"
                mm_layer(f"H{l}", hi_in, lo_in, KP_H, hi_out, lo_out,
                         w_read_hid(l), "wm", gam_sb[l + 1], bet_sb[l + 1],
                         make_lo=use_lo_in(nxt), cc=ccl)

            # ================= output layer + log-softmax ===================
            hi_fin, lo_fin = bufs[n_mid_eff % 2]
            wmo = wmp.tile([128, KP_H, 2, 16], f8, tag="wmo", name="wmo")
            nc.sync.dma_start(
                out=wmo,
                in_=tw_out.rearrange("r klp two p c -> p (r klp) two c"))
            pso = psum.tile([16, B_LOC], f32, tag="mm", name="pso")
            osrcs = [(0, hi_fin)]
            if use_lo_in("out"):
                osrcs.append((1, lo_fin))
            olast = osrcs[-1][0]
            for kp in range(KP_H):
                w_ap = wmo[:, kp, :, :]
                for hl, src in osrcs:
                    for n in range(2):
                        nc.tensor.matmul(
                            pso[:, n * 512:(n + 1) * 512],
                            w_ap,
                            src[kp][:, :, n * 512:(n + 1) * 512],
                            start=(kp == 0 and hl == 0),
                            stop=(kp == KP_H - 1 and hl == olast),
                            perf_mode=PM)
            s12o = stats.tile([10, 2], f32, tag="s12o")
            opre = small.tile([10, B_LOC], f32, tag="opre")
            nc.vector.tensor_scalar(out=opre, in0=pso[0:10, :], scalar1=1.0,
                                    scalar2=None, op0=OP.mult, op1=OP.add,
                                    accum_out=s12o[:, 0:1])
            sjo = ternp.tile([10, B_LOC], bf16, tag="sq", name="sqo", bufs=1)
            nc.scalar.activation(out=sjo, in_=pso[0:10, :], func=AF.Square,
                                 accum_out=s12o[:, 1:2])
            bno_in = dram.tile([10, 2], f32)
            bno_out = dram.tile([10, 2], f32, addr_space="Shared")
            nc.gpsimd.dma_start(out=bno_in, in_=s12o)
            nc.gpsimd.collective_compute(
                "AllReduce", OP.add, replica_groups=RG,
                ins=[bno_in.opt()], outs=[bno_out.opt()])
            sgo = stats.tile([10, 2], f32, tag="sgo")
            nc.gpsimd.dma_start(out=sgo, in_=bno_out)
            meano = stats.tile([10, 1], f32, tag="meano")
            nc.vector.tensor_scalar_mul(meano, sgo[:, 0:1], 1.0 / B)
            ex2o = stats.tile([10, 1], f32, tag="ex2o")
            nc.vector.tensor_scalar_mul(ex2o, sgo[:, 1:2], 1.0 / B)
            msqo = stats.tile([10, 1], f32, tag="msqo")
            nc.vector.tensor_tensor(out=msqo, in0=meano, in1=meano, op=OP.mult)
            varo = stats.tile([10, 1], f32, tag="varo")
            nc.vector.tensor_tensor(out=varo, in0=ex2o, in1=msqo,
                                    op=OP.subtract)
            sdo = stats.tile([10, 1], f32, tag="sdo")
            nc.scalar.activation(out=sdo, in_=varo, func=AF.Sqrt,
                                 bias=epsb[0:10, :])
            rso = stats.tile([10, 1], f32, tag="rso")
            nc.vector.reciprocal(out=rso, in_=sdo)
            sclo = stats.tile([10, 1], f32, tag="sclo")
            nc.vector.tensor_tensor(out=sclo, in0=rso, in1=go_sb, op=OP.mult)
            mso = stats.tile([10, 1], f32, tag="mso")
            nc.vector.tensor_tensor(out=mso, in0=meano, in1=sclo, op=OP.mult)
            biao = stats.tile([10, 1], f32, tag="biao")
            nc.vector.tensor_tensor(out=biao, in0=bo_sb, in1=mso,
                                    op=OP.subtract)
            onorm = opre
            nc.scalar.activation(out=onorm, in_=opre, func=AF.Identity,
                                 bias=biao, scale=sclo)
            esb = small.tile([10, B_LOC], f32, tag="esb")
            nc.scalar.activation(out=esb, in_=onorm, func=AF.Exp)
            csp = psum.tile([1, B_LOC], f32, tag="cs", bufs=1)
            for n in range(2):
                nc.tensor.matmul(csp[:, n * 512:(n + 1) * 512], ones10,
                                 esb[:, n * 512:(n + 1) * 512],
                                 start=True, stop=True)
            lsb = small.tile([1, B_LOC], f32, tag="lsb")
            nc.scalar.activation(out=lsb, in_=csp, func=AF.Ln)
            lse_bc = psum.tile([10, B_LOC], f32, tag="cs", name="lse_bc",
                               bufs=1)
            for n in range(2):
                nc.tensor.matmul(lse_bc[:, n * 512:(n + 1) * 512], ones1x10,
                                 lsb[:, n * 512:(n + 1) * 512],
                                 start=True, stop=True)
            fout = small.tile([10, B_LOC], f32, tag="esb", name="fout")
            nc.vector.tensor_tensor(out=fout, in0=onorm, in1=lse_bc,
                                    op=OP.subtract)
            nc.gpsimd.dma_start(out=out[:], in_=fout)

    nc.compile()
    return nc


def _get_program():
    if "nc" not in _cache:
        _cache["nc"] = _build()
    return _cache["nc"]


def kernel(x, W_in, b_in, W_hid, b_hid, W_out, b_out, gamma, beta,
           gamma_out, beta_out):
    from concourse.bass_utils import run_bass_kernel_spmd

    nc = _get_program()

    x = np.asarray(x, dtype=np.float32).reshape(B, KIN)
    # layout-only host prep (transpose + zero-pad + shard)
    xT_full = np.zeros((KIN_PAD, B), dtype=np.float32)
    xT_full[:KIN] = x.T
    winT_full = np.zeros((KIN_PAD, HID), dtype=np.float32)
    winT_full[:KIN] = np.asarray(W_in, dtype=np.float32).T
    whT_full = np.ascontiguousarray(
        np.asarray(W_hid, dtype=np.float32).transpose(0, 2, 1))
    woT_full = np.zeros((HID, 16), dtype=np.float32)
    woT_full[:, :10] = np.asarray(W_out, dtype=np.float32).T
    gam_np = np.ascontiguousarray(np.asarray(gamma, dtype=np.float32))
    bet_np = np.ascontiguousarray(np.asarray(beta, dtype=np.float32))
    gob_np = np.stack([np.asarray(gamma_out, dtype=np.float32),
                       np.asarray(beta_out, dtype=np.float32)])

    in_maps = []
    for c in range(N_CORES):
        in_maps.append({
            "xT": np.ascontiguousarray(
                xT_full[:, c * B_LOC:(c + 1) * B_LOC]),
            "winT": winT_full,
            "whT": np.ascontiguousarray(
                whT_full[:, c * SH_H:(c + 1) * SH_H, :]),
            "woT": np.ascontiguousarray(
                woT_full[c * SH_H:(c + 1) * SH_H, :]),
            "gam": gam_np,
            "bet": bet_np,
            "gob": gob_np,
        })

    res = run_bass_kernel_spmd(nc, in_maps, core_ids=list(range(N_CORES)))
    return np.concatenate(
        [np.ascontiguousarray(res.results[c]["out"].T) for c in range(N_CORES)],
        axis=0)
